# revision 19
# baseline (speedup 1.0000x reference)
"""Trainium2 Bass kernel for the 3-layer LSTM highway encoder.

Problem: nn_Encoding_layer (B=32, T=512, D=H=512)
  net = lstm1(x)                          # forward LSTM
  net = hw2(net)   = rev-LSTM + highway   # reversed LSTM (per-length) + highway
  net = hw3(net)   = fwd-LSTM + highway

Sharding: data-parallel, batch 32 -> 8 cores x 4 sequences. Weights replicated.

Chunked recurrence: within a core, each layer's T-step recurrence is split
into C=8 time chunks processed SIMULTANEOUSLY as extra batch columns (the
moving operand grows from BC=4 to C*BC=32 columns).  Chunk j covers
t in [j*L+W, (j+1)*L+W) (chunk 0: [0, L+W)) with L = (T-W)/C; every chunk
starts from zero state and runs L+W steps, the first W being warmup whose
outputs are later overwritten by the owning chunk (the LSTM state is
contractive, so after W steps the zero-init error is below threshold; W is
per-layer, larger for layers 2/3 whose state decays slower).  At step s
chunk j processes t = j*L + s (reverse layer: t = j*L + (L+W-1-s)), so the
per-step gather across chunks is one stepped dynamic slice of stride L.
This cuts serial steps ~3x and amortizes all per-step fixed costs 8x.

Device design (per core, everything SBUF-resident, bf16 matmuls / fp32 state):
  - Activations live transposed: [128 d-partitions, 4 d-chunks, PAD+T*4+PAD cols]
    column index = t*4 + b. Zero pads at both ends make the t=0 (forward) and
    t=T-1 (backward) steps read zero recurrent state with no special cases.
  - Phase A (per layer): xg = x @ Wx + b as 256 dense matmuls (Wx stationary
    tiles, activations moving), bias folded via ACT Identity, output bf16
    "xgt" [128, 16 gate-chunks, T*4], gate-chunk order is group/gate-major.
  - Phase B (per layer): T sequential steps in a For_i hardware loop.
    Per step: 64 matmuls (Wh [128,128] bf16 stationary tiles, moving h_{t-1}
    [128,4]) accumulating into per-group PSUM; vector tail computes
    c = sig(f)*c + sig(i)*tanh(j), h = sig(o)*tanh(c).  h lives in a static
    ping-pong buffer (hb) so every matmul AP is static (no per-instruction
    pointer-update sequencer work); GpSimd archives h into the time-indexed
    output buffer.  Matmuls are group-major so group 0's tail hides under
    group 1's matmuls and group 1's tail under the next step's group-0 ones.
    Layer 2 runs t backwards and multiplies c by a host-built (t<len) mask,
    which reproduces tf.reverse_sequence + dynamic_rnn masking exactly.
  - Phase C (layers 2,3): highway gate tg = sigmoid([prev_h, x] @ Wt + bt),
    out = y*tg + (x@Wc)*(1-tg). prev_h is just a 4-column-shifted slice.
  - Final (t >= len) output masking is done on host.
"""

import os

import ml_dtypes
import numpy as np

BF16 = ml_dtypes.bfloat16

# ---------------------------------------------------------------- constants
B, D, H = 32, 512, 512
T = int(os.environ.get("BASSLSTM_T", "512"))
NCORES = 8
BC = B // NCORES            # 4 sequences per core
P = 128
KC = D // P                 # 4 d-chunks
GC = 4 * H // P             # 16 gate chunks
NG = 2                      # hidden-chunk groups in the recurrence tail
S = KC // NG                # hidden chunks per group
GPG = GC // NG              # gate chunks per group
TB = T * BC                 # columns per d-chunk

# --- chunked recurrence ---
# Uniform chunks: C chunks of L = T/C time steps; storage column for time t
# is col(t) = (t mod L)*NCB + (t//L)*BC + b  ("chunk-interleaved" layout),
# so the per-step gather/archive across all C chunks is ONE contiguous
# NCB-column slice.  Each chunk runs W warmup steps before its L owned
# steps; chunks whose warmup window crosses t=0 (fwd; t=T-1 rev) get their
# state zeroed exactly when crossing, making them exact.
C = 8                       # time chunks per core (extra batch columns)
NCB = C * BC                # moving columns per recurrence step
L = T // C                  # owned steps per chunk
PAD = NCB                   # pad columns in activation buffers
PADT = PAD + TB + PAD
NSZ = min(512, TB)          # moving free-dim per phase-A/C matmul
NCH = TB // NSZ             # tb chunks
U = 8                       # step unroll inside For_i

_WDEF = (32, 192, 192) if T == 512 else (T // 8, T // 4, T // 4)
WARMUP = tuple(
    max(U, (int(os.environ.get(f"BASSLSTM_W{i+1}", _WDEF[i])) // U) * U)
    for i in range(3))

# g' (gate-chunk) order: groups of hidden chunks, gate-major inside a group:
# [j j .. | i i .. | f f .. | o o ..] per group.  orig TF gate order: i,j,f,o.
_GATES = (1, 0, 2, 3)       # j, i, f, o -> index into the 4H axis


def _gprime_table():
    tbl = []
    for g in range(NG):
        for go in _GATES:
            for s in range(S):
                tbl.append((go, g * S + s))
    return tbl


# ---------------------------------------------------------------- program
_PROG = None


def _build_program():
    import concourse.mybir as mybir
    import concourse.tile as tile
    from concourse import bacc
    from concourse.bass import ds

    F32 = mybir.dt.float32
    BF = mybir.dt.bfloat16
    AF = mybir.ActivationFunctionType
    OP = mybir.AluOpType

    nc = bacc.Bacc("TRN2", target_bir_lowering=False, debug=False,
                   num_devices=NCORES)

    x0_d = nc.dram_tensor("x0t", [P, KC, PADT], BF, kind="ExternalInput")
    wx_d = nc.dram_tensor("wx", [3, P, KC, GC, P], BF, kind="ExternalInput")
    wh_d = nc.dram_tensor("wh", [3, P, KC, GC, P], BF, kind="ExternalInput")
    wt_d = nc.dram_tensor("wt", [2, P, 2 * KC, KC, P], BF, kind="ExternalInput")
    wc_d = nc.dram_tensor("wc", [2, P, KC, KC, P], BF, kind="ExternalInput")
    bias_d = nc.dram_tensor("bias", [3, P, GC], F32, kind="ExternalInput")
    bt_d = nc.dram_tensor("bt", [2, P, KC], F32, kind="ExternalInput")
    # mask planes (chunk-interleaved cols): m in {0,1}; q = (m-1)*15
    mm_d = nc.dram_tensor("maskm", [P, PADT], BF, kind="ExternalInput")
    mq_d = nc.dram_tensor("maskq", [P, PADT], BF, kind="ExternalInput")
    id_d = nc.dram_tensor("ident", [P, P], BF, kind="ExternalInput")
    out_d = nc.dram_tensor("outt", [P, KC, TB], F32, kind="ExternalOutput")

    with tile.TileContext(nc) as tc:
        with (
            tc.tile_pool(name="per", bufs=1) as per,
            tc.tile_pool(name="wpool", bufs=1) as wpool,
            tc.tile_pool(name="work", bufs=6) as work,
            tc.tile_pool(name="hwork", bufs=3) as hwork,
            tc.tile_pool(name="psb", bufs=5, space="PSUM") as psb,
            tc.tile_pool(name="psbig", bufs=3, space="PSUM") as psbig,
        ):
            # Activation buffers are split into NG parts (hidden-chunk
            # groups).  Cross-step deps through these tensors use dynamic
            # APs, which Tile tracks at whole-tensor granularity — the split
            # makes next step's k<S matmuls depend only on group 0's h-write,
            # hiding group 1's vector tail under the next step's matmuls.
            def mkbuf(name):
                return tuple(
                    per.tile([P, S, PADT], BF, name=f"{name}_{g}")
                    for g in range(NG))

            buf1 = mkbuf("buf1")
            buf2 = mkbuf("buf2")
            buf3 = mkbuf("buf3")
            xgt = per.tile([P, GC, PADT], BF)
            biasb = per.tile([P, 3 * GC], F32)
            btb = per.tile([P, 2 * KC], F32)
            maskm = per.tile([P, PADT], BF)
            maskq = per.tile([P, PADT], BF)
            identb = per.tile([P, P], BF)
            c = per.tile([P, KC, NCB], F32)

            # initial loads
            for g in range(NG):
                nc.sync.dma_start(buf1[g][:], x0_d[:, g * S:(g + 1) * S, :])
            for l in range(3):
                nc.sync.dma_start(biasb[:, l * GC:(l + 1) * GC], bias_d[l])
            for li in range(2):
                nc.sync.dma_start(btb[:, li * KC:(li + 1) * KC], bt_d[li])
            nc.sync.dma_start(maskm[:], mm_d[:])
            nc.sync.dma_start(maskq[:], mq_d[:])
            nc.sync.dma_start(identb[:], id_d[:])
            # zero pads of the two reusable buffers (buf1 pads come from host)
            for buf in (buf2, buf3):
                for g in range(NG):
                    nc.vector.memset(buf[g][:, :, 0:PAD], 0.0)
                    nc.vector.memset(buf[g][:, :, PAD + TB:], 0.0)
            # xgt pads: warmup steps read (junk) columns there; keep finite
            nc.vector.memset(xgt[:, :, 0:PAD], 0.0)
            nc.vector.memset(xgt[:, :, PAD + TB:], 0.0)

            def bk(buf, k):
                """(part, local chunk index) for global d-chunk k."""
                return buf[k // S], k % S

            def load_w(pool, src, shape, tag):
                t_ = pool.tile(shape, BF, tag=tag)
                nc.sync.dma_start(t_[:], src)
                return t_

            def phase_xg(l, xin, wxb):
                for gp in range(GC):
                    for n in range(NCH):
                        psum = psbig.tile([P, NSZ], F32, tag="big")
                        for k in range(KC):
                            xpart, kl = bk(xin, k)
                            nc.tensor.matmul(
                                psum[:], wxb[:, k, gp, :],
                                xpart[:, kl, PAD + n * NSZ:PAD + (n + 1) * NSZ],
                                start=(k == 0), stop=(k == KC - 1))
                        nc.scalar.activation(
                            xgt[:, gp, PAD + n * NSZ:PAD + (n + 1) * NSZ],
                            psum[:],
                            AF.Identity, bias=biasb[:, l * GC + gp:l * GC + gp + 1])

            def fold_mask():
                # Fold the (t < len) mask of the reversed layer into its i/f
                # gate pre-activations: x' = x*m + (m-1)*15, i.e. unmasked
                # columns keep x, masked columns become -15 (= 0.5*(-30)),
                # which saturates sig() to exactly 0 in the tail: c and h
                # stay exactly 0 through masked steps -- replaces the
                # per-step c-mask multiply of the recurrence.
                for rows in (slice(2, 6), slice(10, 14)):
                    seg = xgt[:, rows, :]
                    mb = maskm.unsqueeze(1).to_broadcast([P, 4, PADT])
                    qb = maskq.unsqueeze(1).to_broadcast([P, 4, PADT])
                    nc.vector.tensor_tensor(seg, seg, mb, OP.mult)
                    nc.vector.tensor_tensor(seg, seg, qb, OP.add)

            MMORD = int(os.environ.get("BASSLSTM_MMORD", "0"))

            def mm_order():
                # The per-step dependency cycle is h_g(t-1) -> MMs gated by
                # it -> psum complete -> tail chain -> h_g(t).  MMORD=1:
                # group-0's full psum completes after 32 MMs (k<2 for its
                # gate chunks first, then k>=2 which need group-1's h of the
                # previous step -- ready earlier than group-0's h), so its
                # tail starts while group-1's MMs stream.  MMORD=0: all k<2
                # first (baseline order).
                if MMORD == 2:     # group-major
                    return ([(gp, k) for g in range(NG) for k in range(KC)
                             for gp in range(g * GPG, (g + 1) * GPG)],
                            (0, 1))
                if MMORD == 1:
                    order = []
                    for g in range(NG):
                        order += [(gp, k) for k in range(KC // 2)
                                  for gp in range(g * GPG, (g + 1) * GPG)]
                        order += [(gp, k) for k in range(KC // 2, KC)
                                  for gp in range(g * GPG, (g + 1) * GPG)]
                    return order, (0, 1)
                first = [(gp, k) for k in range(KC // 2) for gp in range(GC)]
                rest = []
                for g in (0, 1):
                    rest += [(gp, k)
                             for gp in range(g * GPG, (g + 1) * GPG)
                             for k in range(KC // 2, KC)]
                return first + rest, (0, 1)

            MM_ORDER, TAIL_ORDER = mm_order()

            GW = KC * BC  # mask columns per time step
            NOMM = os.environ.get("BASSLSTM_NOMM") == "1"
            NOTAIL = os.environ.get("BASSLSTM_NOTAIL") == "1"
            WARM = int(os.environ.get("BASSLSTM_WARM", "0"))
            TAILPRIO = os.environ.get("BASSLSTM_TAILPRIO", "1") == "1"

            import contextlib

            def tail_prio(l, u, gi):
                # Pseudo-time floor so the Tile scheduler lays the tail ops
                # out on the DVE/ACT FIFOs strictly in (step, group) order;
                # otherwise group 1's psum-add (ready only at stream end)
                # gets queued AHEAD of group 0's remaining chain and blocks
                # it (in-order engine queues).
                if not TAILPRIO:
                    return contextlib.nullcontext()
                return tc.tile_wait_until((l * 1000 + u * NG + gi) * 0.01)

            # h ping-pong: static-address feedback buffers so the matmul
            # stream has NO dynamic APs (kills the per-instruction pointer-
            # update sequencer work).  Split per (parity, group) for precise
            # dependency tracking.  The time-indexed output sequence is
            # archived separately on the (otherwise idle) GpSimd engine.
            hb = [[per.tile([P, S, NCB], BF, name=f"hb{p_}_{g_}")
                   for g_ in range(NG)] for p_ in range(2)]
            # GpSimd prefetches the NEXT step's xg slice (contiguous NCB
            # columns in the interleaved layout) into a static ping-pong
            # buffer; the identity matmul injects it into PSUM.
            xgb = [per.tile([P, GC, NCB], BF, name=f"xgb{p_}")
                   for p_ in range(2)]

            def rec_step(l, yout, rev, whb, W, s_expr, s_next, u, archive):
                """One recurrence step.  s_expr: affine loop expr for the
                xg/archive column base of THIS step (already in column
                units, PAD included); s_next: same for the NEXT step's
                prefetch; u: unroll parity index."""
                rd = hb[u % 2]          # h(t-1), static APs
                wr = hb[(u + 1) % 2]    # h(t)
                par = u % 2
                npar = (u + 1) % 2
                # prefetch next step's xg slice (consumed by next ident-MM)
                nc.gpsimd.tensor_copy(
                    xgb[npar][:], xgt[:, :, ds(s_next, NCB)])
                pss = [psb.tile([P, GPG, NCB], F32, tag="psb",
                                name=f"ps{g_}")
                       for g_ in range(NG)]
                # identity matmul seeds each group's psum with xg (+bias),
                # so the tail's tanh reads PSUM directly (no DVE add).
                for g_ in range(NG):
                    nc.tensor.matmul(
                        pss[g_][:], identb[:],
                        xgb[par][:, g_ * GPG:(g_ + 1) * GPG, :],
                        start=True, stop=False, skip_group_check=True)
                for pi, (gp, k) in enumerate([] if NOMM else MM_ORDER):
                    g, gl = divmod(gp, GPG)
                    rhs = rd[k // S][:, k % S, :]
                    nc.tensor.matmul(
                        pss[g][:, gl, :], whb[:, k, gp, :],
                        rhs,
                        start=False,
                        stop=(k == KC - 1),
                        skip_group_check=True)
                # Tail per group (state is tracked doubled: the hb
                # buffers hold h2 = 2h — compensated by an extra
                # 0.5 folded into Wh host-side — and c holds C' = 2c):
                #   th   = tanh(psum')          (j full, i/f/o half)
                #   t1'  = (th_i + 1) * th_j    = 2*sig_i*tanh_j
                #   u    = (th_f + 1) * C'      = 4*sig_f*c
                #   C'   = 0.5*u + t1'          = 2*c_new
                #   tct  = tanh(0.5*C')         = tanh(c_new)
                #   h2   = (th_o + 1) * tct     = 2*h
                for gi, g in enumerate([] if NOTAIL else TAIL_ORDER):
                  with tail_prio(l, u, gi):
                    th = work.tile([P, 4 * S, NCB], F32, tag="th")
                    nc.scalar.activation(th[:], pss[g][:], AF.Tanh)
                    t1 = work.tile([P, S, NCB], F32, tag="t1")
                    nc.vector.scalar_tensor_tensor(
                        t1[:], th[:, S:2 * S, :], 1.0, th[:, 0:S, :],
                        OP.add, OP.mult)
                    cg = c[:, g * S:(g + 1) * S, :]
                    uu = work.tile([P, S, NCB], F32, tag="uu")
                    nc.vector.scalar_tensor_tensor(
                        uu[:], th[:, 2 * S:3 * S, :], 1.0, cg,
                        OP.add, OP.mult)
                    nc.vector.scalar_tensor_tensor(
                        cg, uu[:], 0.5, t1[:], OP.mult, OP.add)
                    tct = work.tile([P, S, NCB], F32, tag="tct")
                    nc.scalar.activation(tct[:], cg, AF.Tanh,
                                         scale=0.5)
                    nc.vector.scalar_tensor_tensor(
                        wr[g][:], th[:, 3 * S:4 * S, :], 1.0, tct[:],
                        OP.add, OP.mult)
                    if archive:
                        # archive h2(t) = 2*h(t): contiguous NCB columns
                        # in the interleaved layout, via the (idle) DMA
                        # engines -- GpSimd dynamic-dest copies cost ~1.9us
                        # and stall concurrent DVE ops (port contention).
                        # yout holds DOUBLED values; consumers compensate
                        # (weight scales host-side + 0.5 in the highway
                        # STT).
                        nc.sync.dma_start(
                            yout[g][:, :, ds(s_expr, NCB)], wr[g][:])

            def zero_chunk(jz):
                """Exact reset of chunk jz's state (its t crossed the
                sequence boundary): zero its c and h columns."""
                sl = slice(jz * BC, (jz + 1) * BC)
                nc.vector.memset(c[:, :, sl], 0.0)
                for g_ in range(NG):
                    nc.vector.memset(hb[0][g_][:, :, sl], 0.0)

            def phase_rec(l, yout, whb, rev):
                W = WARMUP[l]
                nc.vector.memset(c[:], 0.0)
                for g_ in range(NG):
                    nc.vector.memset(hb[0][g_][:], 0.0)

                def colbase(s):
                    # column base (incl PAD) of step s's gather/archive:
                    # delta = s-W (fwd) or L-1-(s-W) (rev); the C chunks'
                    # columns are contiguous: PAD + (delta%L)*NCB +
                    # (delta//L)*BC, with delta//L constant inside a block.
                    delta = (s - W) if not rev else (L - 1 - (s - W))
                    return PAD + (delta % L) * NCB + (delta // L) * BC

                blocks = ([W % L] if W % L else []) + [L] * (W // L) + [L]
                s0 = 0
                for bi, blk in enumerate(blocks):
                    archive = (bi == len(blocks) - 1)
                    # prologue prefetch of this block's first step (static)
                    nc.gpsimd.tensor_copy(
                        xgb[0][:], xgt[:, :, colbase(s0):colbase(s0) + NCB])
                    sgn = 1 if not rev else -1
                    base0 = colbase(s0)
                    with tc.For_i(0, blk // U, 1,
                                  hint_engines=(mybir.EngineType.PE,)) as i:
                        for u in range(U):
                            # affine column exprs for step s = s0 + i*U + u
                            sexp = i * (sgn * U * NCB) + (
                                base0 + sgn * u * NCB)
                            snxt = i * (sgn * U * NCB) + (
                                base0 + sgn * (u + 1) * NCB)
                            rec_step(l, yout, rev, whb, W,
                                     sexp, snxt, u, archive)
                    s0 += blk
                    if s0 <= W and (W - s0) % L == 0 and (W - s0) // L < C:
                        jz = (W - s0) // L
                        zero_chunk(jz if not rev else C - 1 - jz)
                # fill yout's front pad with y(j*L - 1) for the highway
                # prev_h shift: block (L-1, j-1) -> pad cols [BC, NCB)
                # (j=0 keeps zeros = prev_h at t=0).
                for g_ in range(NG):
                    nc.gpsimd.tensor_copy(
                        yout[g_][:, :, BC:NCB],
                        yout[g_][:, :, PAD + (L - 1) * NCB:
                                 PAD + (L - 1) * NCB + (NCB - BC)])

            def phase_hw(li, y, x, out_sbuf, wtb, wcb):
                for gc_ in range(KC):
                    for n in range(NCH):
                        pt = psbig.tile([P, NSZ], F32, tag="big")
                        for k in range(KC):
                            yp, kl = bk(y, k)
                            nc.tensor.matmul(
                                pt[:], wtb[:, k, gc_, :],
                                yp[:, kl,
                                   PAD + n * NSZ - NCB:PAD + (n + 1) * NSZ - NCB],
                                start=(k == 0), stop=False)
                        for k in range(KC):
                            xp, kl = bk(x, k)
                            nc.tensor.matmul(
                                pt[:], wtb[:, KC + k, gc_, :],
                                xp[:, kl, PAD + n * NSZ:PAD + (n + 1) * NSZ],
                                start=False, stop=(k == KC - 1))
                        tg = hwork.tile([P, NSZ], BF, tag="tg")
                        nc.scalar.activation(
                            tg[:], pt[:], AF.Sigmoid,
                            bias=btb[:, li * KC + gc_:li * KC + gc_ + 1])
                        pc = psbig.tile([P, NSZ], F32, tag="big")
                        for k in range(KC):
                            xp, kl = bk(x, k)
                            nc.tensor.matmul(
                                pc[:], wcb[:, k, gc_, :],
                                xp[:, kl, PAD + n * NSZ:PAD + (n + 1) * NSZ],
                                start=(k == 0), stop=(k == KC - 1))
                        dt_ = hwork.tile([P, NSZ], F32, tag="dt")
                        ypart, ykl = bk(y, gc_)
                        # y buffers hold 2*h; fold the 0.5 into the fused op
                        nc.vector.scalar_tensor_tensor(
                            dt_[:], ypart[:, ykl, PAD + n * NSZ:PAD + (n + 1) * NSZ],
                            0.5, pc[:], OP.mult, OP.subtract)
                        nc.vector.tensor_tensor(dt_[:], dt_[:], tg[:], OP.mult)
                        if out_sbuf is not None:
                            opart, okl = bk(out_sbuf, gc_)
                            nc.vector.tensor_tensor(
                                opart[:, okl, PAD + n * NSZ:PAD + (n + 1) * NSZ],
                                dt_[:], pc[:], OP.add)
                        else:
                            st = hwork.tile([P, NSZ], F32, tag="st")
                            nc.vector.tensor_tensor(st[:], dt_[:], pc[:], OP.add)
                            nc.sync.dma_start(
                                out_d[:, gc_, n * NSZ:(n + 1) * NSZ], st[:])

            NL = int(os.environ.get("BASSLSTM_LAYERS", "3"))

            # ---- layer 1 (plain forward LSTM)
            wxb = load_w(wpool, wx_d[0], [P, KC, GC, P], "wx")
            whb = load_w(wpool, wh_d[0], [P, KC, GC, P], "wh")
            phase_xg(0, buf1, wxb)
            phase_rec(0, buf2, whb, rev=False)
            for _rep in range(int(os.environ.get("BASSLSTM_REPEAT", "1")) - 1):
                phase_rec(0, buf2, whb, rev=False)
            if NL == 1:
                for gc_ in range(KC):
                    for n in range(NCH):
                        st0 = hwork.tile([P, NSZ], F32, tag="st", name="st0")
                        bp, bkl = bk(buf2, gc_)
                        nc.vector.tensor_scalar_mul(
                            st0[:],
                            bp[:, bkl, PAD + n * NSZ:PAD + (n + 1) * NSZ],
                            0.5)
                        nc.sync.dma_start(
                            out_d[:, gc_, n * NSZ:(n + 1) * NSZ], st0[:])
            else:
                wxb2 = load_w(wpool, wx_d[1], [P, KC, GC, P], "wx")
                wtb = load_w(wpool, wt_d[0], [P, 2 * KC, KC, P], "wt")
                wcb = load_w(wpool, wc_d[0], [P, KC, KC, P], "wc")
                whb2 = load_w(wpool, wh_d[1], [P, KC, GC, P], "wh")

                # ---- layer 2 (reversed LSTM + highway)
                phase_xg(1, buf2, wxb2)
                fold_mask()
                phase_rec(1, buf3, whb2, rev=True)
                if NL == 2:
                    phase_hw(0, buf3, buf2, None, wtb, wcb)
                else:
                    wxb3 = load_w(wpool, wx_d[2], [P, KC, GC, P], "wx")
                    whb3 = load_w(wpool, wh_d[2], [P, KC, GC, P], "wh")
                    phase_hw(0, buf3, buf2, buf1, wtb, wcb)
                    wtb2 = load_w(wpool, wt_d[1], [P, 2 * KC, KC, P], "wt")
                    wcb2 = load_w(wpool, wc_d[1], [P, KC, KC, P], "wc")

                    # ---- layer 3 (forward LSTM + highway -> DRAM)
                    phase_xg(2, buf1, wxb3)
                    phase_rec(2, buf2, whb3, rev=False)
                    phase_hw(1, buf2, buf1, None, wtb2, wcb2)

    nc.compile()
    return nc


def _program():
    global _PROG
    if _PROG is None:
        _PROG = _build_program()
    return _PROG


# ---------------------------------------------------------------- host side
def _prep_weights(inp):
    """Build the shared (replicated) weight arrays in device layout."""
    gtbl = _gprime_table()
    wx = np.zeros((3, P, KC, GC, P), np.float32)
    wh = np.zeros((3, P, KC, GC, P), np.float32)
    bias = np.zeros((3, P, GC), np.float32)
    for l, (wxn, whn, bn) in enumerate(
            [("Wx1", "Wh1", "b1"), ("Wx2", "Wh2", "b2"), ("Wx3", "Wh3", "b3")]):
        Wx = np.asarray(inp[wxn], np.float32)
        Wh = np.asarray(inp[whn], np.float32)
        b = np.asarray(inp[bn], np.float32)
        for gp, (go, m) in enumerate(gtbl):
            cs = go * H + m * P
            # sigmoid gates (i,f,o) are computed as (tanh(x/2)+1)/2 on
            # device so one ACT op covers all four gates; fold the /2 into
            # the weights and bias here.  j (go==1) stays unscaled.  Wh
            # gets an extra 0.5 because the recurrent h buffer holds 2h.
            # Wx of layer 2 gets 0.5 too: its input (layer 1's archived
            # output) is doubled; layer 3's input is a highway output
            # (true scale).
            s_ = 1.0 if go == 1 else 0.5
            xs_ = 0.5 if l == 1 else 1.0
            for k in range(KC):
                wx[l, :, k, gp, :] = Wx[k * P:(k + 1) * P, cs:cs + P] * (
                    s_ * xs_)
                wh[l, :, k, gp, :] = Wh[k * P:(k + 1) * P, cs:cs + P] * (
                    s_ * 0.5)
            bias[l, :, gp] = b[cs:cs + P]
            if go == 2:  # forget gate: fold forget_bias = 1.0
                bias[l, :, gp] += 1.0
            bias[l, :, gp] *= s_
    wt = np.zeros((2, P, 2 * KC, KC, P), np.float32)
    wc = np.zeros((2, P, KC, KC, P), np.float32)
    bt = np.zeros((2, P, KC), np.float32)
    for li, (wtn, wcn, btn) in enumerate(
            [("Wt2", "Wc2", "bt2"), ("Wt3", "Wc3", "bt3")]):
        Wt = np.asarray(inp[wtn], np.float32)
        Wc = np.asarray(inp[wcn], np.float32)
        btv = np.asarray(inp[btn], np.float32)
        for gc_ in range(KC):
            cs = gc_ * P
            for k in range(2 * KC):
                # rows [0,KC): prev_h operand = this layer's archived LSTM
                # output (doubled) -> 0.5.  rows [KC,2KC): the x operand —
                # doubled for hw2 (x = layer-1 archive), true for hw3
                # (x = hw2's highway output).
                ts_ = 0.5 if (k < KC or li == 0) else 1.0
                wt[li, :, k, gc_, :] = Wt[k * P:(k + 1) * P, cs:cs + P] * ts_
            for k in range(KC):
                cs_ = 0.5 if li == 0 else 1.0
                wc[li, :, k, gc_, :] = Wc[k * P:(k + 1) * P, cs:cs + P] * cs_
            bt[li, :, gc_] = btv[cs:cs + P]
    return (wx.astype(BF16), wh.astype(BF16), wt.astype(BF16),
            wc.astype(BF16), bias, bt)


def _interleave_cols(arr_t_last):
    """[..., T, BC] -> [..., TB] in chunk-interleaved column order
    col(t=j*L+lt, b) = lt*NCB + j*BC + b."""
    shp = arr_t_last.shape[:-2]
    a = arr_t_last.reshape(*shp, C, L, BC)
    a = np.moveaxis(a, -3, -2)               # [..., L, C, BC]
    return np.ascontiguousarray(a).reshape(*shp, TB)


def _host_prep(inputs):
    x = np.asarray(inputs["inputs"], np.float32)
    length = np.asarray(inputs["length"], np.int32)
    wx, wh, wt, wc, bias, bt = _prep_weights(inputs)
    ident = np.eye(P, dtype=BF16)
    in_maps = []
    for ci in range(NCORES):
        xc = x[ci * BC:(ci + 1) * BC, :T]          # [BC, T, D]
        arr = np.ascontiguousarray(xc.transpose(2, 1, 0))  # [D, T, BC]
        arr = arr.reshape(KC, P, T, BC).transpose(1, 0, 2, 3)  # [P, KC, T, BC]
        x0t = np.zeros((P, KC, PADT), BF16)
        x0t[:, :, PAD:PAD + TB] = _interleave_cols(arr).astype(BF16)
        lc = length[ci * BC:(ci + 1) * BC]
        m = (np.arange(T)[:, None] < lc[None, :]).astype(np.float32)  # [T, BC]
        mi = _interleave_cols(m)                   # [TB]
        maskm = np.zeros((PADT,), np.float32)
        maskm[PAD:PAD + TB] = mi
        maskq = np.zeros((PADT,), np.float32)
        maskq[PAD:PAD + TB] = (mi - 1.0) * 15.0
        in_maps.append({
            "x0t": x0t,
            "wx": wx, "wh": wh, "wt": wt, "wc": wc,
            "bias": bias, "bt": bt,
            "maskm": np.broadcast_to(maskm, (P, PADT)).astype(BF16),
            "maskq": np.broadcast_to(maskq, (P, PADT)).astype(BF16),
            "ident": ident,
        })
    return in_maps


def _host_post(results, inputs):
    length = np.asarray(inputs["length"], np.int32)
    out = np.zeros((B, T, D), np.float32)
    for ci, res in enumerate(results):
        o = res["outt"]                      # [P, KC, TB] interleaved cols
        o = o.reshape(P, KC, L, C, BC)
        o = o.transpose(4, 3, 2, 1, 0)       # [BC, C, L, KC, P]
        out[ci * BC:(ci + 1) * BC] = o.reshape(BC, T, D)
    tmask = np.arange(T)[None, :] < length[:, None]
    out *= tmask[:, :, None]
    return out


_RUNNER = None


def _make_runner():
    """Build a cached shard_map-jitted executable for the 8-core program.

    Modeled on concourse.bass2jax.run_bass_via_pjrt, but reusable across
    calls and able to take pre-placed (device-resident) inputs so pure
    execution can be timed without host->device transfer.
    """
    import jax
    import numpy as jnp_np  # noqa: F401
    import concourse.mybir as mybir
    from concourse import bass2jax
    from jax.sharding import Mesh, PartitionSpec
    from jax.experimental.shard_map import shard_map

    nc = _program()
    bass2jax.install_neuronx_cc_hook()

    partition_name = (nc.partition_id_tensor.name
                      if nc.partition_id_tensor else None)
    in_names, out_names, out_avals, zero_outs = [], [], [], []
    for alloc in nc.m.functions[0].allocations:
        if not isinstance(alloc, mybir.MemoryLocationSet):
            continue
        name = alloc.memorylocations[0].name
        if alloc.kind == "ExternalInput":
            if name != partition_name:
                in_names.append(name)
        elif alloc.kind == "ExternalOutput":
            shape = tuple(alloc.tensor_shape)
            dtype = mybir.dt.np(alloc.dtype)
            out_names.append(name)
            out_avals.append(jax.core.ShapedArray(shape, dtype))
            zero_outs.append(np.zeros(shape, dtype))
    n_params = len(in_names)
    all_names = in_names + out_names
    if partition_name is not None:
        all_names.append(partition_name)

    def _body(*args):
        operands = list(args)
        if partition_name is not None:
            operands.append(bass2jax.partition_id_tensor())
        outs = bass2jax._bass_exec_p.bind(
            *operands,
            out_avals=tuple(out_avals),
            in_names=tuple(all_names),
            out_names=tuple(out_names),
            lowering_input_output_aliases=(),
            sim_require_finite=True,
            sim_require_nnan=True,
            nc=nc,
        )
        return tuple(outs)

    devices = jax.devices()[:NCORES]
    mesh = Mesh(np.asarray(devices), ("core",))
    n_out = len(out_names)
    sharded = jax.jit(
        shard_map(_body, mesh=mesh,
                  in_specs=(PartitionSpec("core"),) * (n_params + n_out),
                  out_specs=(PartitionSpec("core"),) * n_out,
                  check_rep=False),
        keep_unused=True,
    )
    return {
        "fn": sharded, "in_names": in_names, "out_names": out_names,
        "zero_outs": zero_outs, "n_params": n_params, "mesh": mesh,
    }


def _runner():
    global _RUNNER
    if _RUNNER is None:
        _RUNNER = _make_runner()
    return _RUNNER


def place_inputs(in_maps):
    """Concatenate per-core inputs on axis 0 (shard_map layout)."""
    r = _runner()
    concat = [np.concatenate([np.asarray(in_maps[c][n])
                              for c in range(NCORES)], axis=0)
              for n in r["in_names"]]
    concat += [np.zeros((NCORES * z.shape[0], *z.shape[1:]), z.dtype)
               for z in r["zero_outs"]]
    return concat


def exec_placed_nofetch(placed):
    """Run and block, but leave outputs on device (for timing)."""
    r = _runner()
    outs = r["fn"](*placed)
    for o in outs:
        o.block_until_ready()
    return outs


def exec_placed(placed):
    r = _runner()
    outs = r["fn"](*placed)
    outs = [o.block_until_ready() for o in outs]
    results = []
    for c in range(NCORES):
        m = {}
        for i, name in enumerate(r["out_names"]):
            z = r["zero_outs"][i]
            arr = np.asarray(outs[i])
            m[name] = arr[c * z.shape[0]:(c + 1) * z.shape[0]]
        results.append(m)
    return results


def run_device(in_maps):
    return exec_placed(place_inputs(in_maps))


def kernel(**inputs):
    in_maps = _host_prep(inputs)
    results = run_device(in_maps)
    return _host_post(results, inputs)



# revision 21
# speedup vs baseline: 1.8829x; 1.8829x over previous
"""Trainium2 Bass kernel for the 3-layer LSTM highway encoder.

Problem: nn_Encoding_layer (B=32, T=512, D=H=512)
  net = lstm1(x)                          # forward LSTM
  net = hw2(net)   = rev-LSTM + highway   # reversed LSTM (per-length) + highway
  net = hw3(net)   = fwd-LSTM + highway

Sharding: data-parallel, batch 32 -> 8 cores x 4 sequences. Weights replicated.

Chunked recurrence: within a core, each layer's T-step recurrence is split
into C=8 time chunks processed SIMULTANEOUSLY as extra batch columns (the
moving operand grows from BC=4 to C*BC=32 columns).  Chunk j covers
t in [j*L+W, (j+1)*L+W) (chunk 0: [0, L+W)) with L = (T-W)/C; every chunk
starts from zero state and runs L+W steps, the first W being warmup whose
outputs are later overwritten by the owning chunk (the LSTM state is
contractive, so after W steps the zero-init error is below threshold; W is
per-layer, larger for layers 2/3 whose state decays slower).  At step s
chunk j processes t = j*L + s (reverse layer: t = j*L + (L+W-1-s)), so the
per-step gather across chunks is one stepped dynamic slice of stride L.
This cuts serial steps ~3x and amortizes all per-step fixed costs 8x.

Device design (per core, everything SBUF-resident, bf16 matmuls / fp32 state):
  - Activations live transposed: [128 d-partitions, 4 d-chunks, PAD+T*4+PAD cols]
    column index = t*4 + b. Zero pads at both ends make the t=0 (forward) and
    t=T-1 (backward) steps read zero recurrent state with no special cases.
  - Phase A (per layer): xg = x @ Wx + b as 256 dense matmuls (Wx stationary
    tiles, activations moving), bias folded via ACT Identity, output bf16
    "xgt" [128, 16 gate-chunks, T*4], gate-chunk order is group/gate-major.
  - Phase B (per layer): T sequential steps in a For_i hardware loop.
    Per step: 64 matmuls (Wh [128,128] bf16 stationary tiles, moving h_{t-1}
    [128,4]) accumulating into per-group PSUM; vector tail computes
    c = sig(f)*c + sig(i)*tanh(j), h = sig(o)*tanh(c).  h lives in a static
    ping-pong buffer (hb) so every matmul AP is static (no per-instruction
    pointer-update sequencer work); GpSimd archives h into the time-indexed
    output buffer.  Matmuls are group-major so group 0's tail hides under
    group 1's matmuls and group 1's tail under the next step's group-0 ones.
    Layer 2 runs t backwards and multiplies c by a host-built (t<len) mask,
    which reproduces tf.reverse_sequence + dynamic_rnn masking exactly.
  - Phase C (layers 2,3): highway gate tg = sigmoid([prev_h, x] @ Wt + bt),
    out = y*tg + (x@Wc)*(1-tg). prev_h is just a 4-column-shifted slice.
  - Final (t >= len) output masking is done on host.
"""

import os

import ml_dtypes
import numpy as np

BF16 = ml_dtypes.bfloat16

# ---------------------------------------------------------------- constants
B, D, H = 32, 512, 512
T = int(os.environ.get("BASSLSTM_T", "512"))
NCORES = 8
BC = B // NCORES            # 4 sequences per core
P = 128
KC = D // P                 # 4 d-chunks
GC = 4 * H // P             # 16 gate chunks
NG = 2                      # hidden-chunk groups in the recurrence tail
S = KC // NG                # hidden chunks per group
GPG = GC // NG              # gate chunks per group
TB = T * BC                 # columns per d-chunk

# --- chunked recurrence ---
# Uniform chunks: C chunks of L = T/C time steps; storage column for time t
# is col(t) = (t mod L)*NCB + (t//L)*BC + b  ("chunk-interleaved" layout),
# so the per-step gather/archive across all C chunks is ONE contiguous
# NCB-column slice.  Each chunk runs W warmup steps before its L owned
# steps; chunks whose warmup window crosses t=0 (fwd; t=T-1 rev) get their
# state zeroed exactly when crossing, making them exact.
C = 8                       # time chunks per core (extra batch columns)
NCB = C * BC                # moving columns per recurrence step
L = T // C                  # owned steps per chunk
PAD = NCB                   # pad columns in activation buffers
PADT = PAD + TB + PAD
NSZ = min(512, TB)          # moving free-dim per phase-A/C matmul
NCH = TB // NSZ             # tb chunks
U = 8                       # step unroll inside For_i

_WDEF = (32, 192, 192) if T == 512 else (T // 8, T // 4, T // 4)
WARMUP = tuple(
    max(U, (int(os.environ.get(f"BASSLSTM_W{i+1}", _WDEF[i])) // U) * U)
    for i in range(3))

# g' (gate-chunk) order: groups of hidden chunks, gate-major inside a group:
# [j j .. | i i .. | f f .. | o o ..] per group.  orig TF gate order: i,j,f,o.
_GATES = (1, 0, 2, 3)       # j, i, f, o -> index into the 4H axis


def _gprime_table():
    tbl = []
    for g in range(NG):
        for go in _GATES:
            for s in range(S):
                tbl.append((go, g * S + s))
    return tbl


# ---------------------------------------------------------------- program
_PROG = None


def _build_program():
    import concourse.mybir as mybir
    import concourse.tile as tile
    from concourse import bacc
    from concourse.bass import ds

    F32 = mybir.dt.float32
    BF = mybir.dt.bfloat16
    AF = mybir.ActivationFunctionType
    OP = mybir.AluOpType

    nc = bacc.Bacc("TRN2", target_bir_lowering=False, debug=False,
                   num_devices=NCORES)

    x0_d = nc.dram_tensor("x0t", [P, KC, PADT], BF, kind="ExternalInput")
    wx_d = nc.dram_tensor("wx", [3, P, KC, GC, P], BF, kind="ExternalInput")
    wh_d = nc.dram_tensor("wh", [3, P, KC, GC, P], BF, kind="ExternalInput")
    wt_d = nc.dram_tensor("wt", [2, P, 2 * KC, KC, P], BF, kind="ExternalInput")
    wc_d = nc.dram_tensor("wc", [2, P, KC, KC, P], BF, kind="ExternalInput")
    bias_d = nc.dram_tensor("bias", [3, P, GC], F32, kind="ExternalInput")
    bt_d = nc.dram_tensor("bt", [2, P, KC], F32, kind="ExternalInput")
    # mask planes (chunk-interleaved cols): m in {0,1}; q = (m-1)*15
    mm_d = nc.dram_tensor("maskm", [P, PADT], BF, kind="ExternalInput")
    mq_d = nc.dram_tensor("maskq", [P, PADT], BF, kind="ExternalInput")
    id_d = nc.dram_tensor("ident", [P, P], BF, kind="ExternalInput")
    out_d = nc.dram_tensor("outt", [P, KC, TB], F32, kind="ExternalOutput")

    with tile.TileContext(nc) as tc:
        with (
            tc.tile_pool(name="per", bufs=1) as per,
            tc.tile_pool(name="wpool", bufs=1) as wpool,
            tc.tile_pool(name="work", bufs=6) as work,
            tc.tile_pool(name="hwork", bufs=3) as hwork,
            tc.tile_pool(name="psb", bufs=5, space="PSUM") as psb,
            tc.tile_pool(name="psbig", bufs=3, space="PSUM") as psbig,
        ):
            # Activation buffers are split into NG parts (hidden-chunk
            # groups).  Cross-step deps through these tensors use dynamic
            # APs, which Tile tracks at whole-tensor granularity — the split
            # makes next step's k<S matmuls depend only on group 0's h-write,
            # hiding group 1's vector tail under the next step's matmuls.
            def mkbuf(name):
                return tuple(
                    per.tile([P, S, PADT], BF, name=f"{name}_{g}")
                    for g in range(NG))

            buf1 = mkbuf("buf1")
            buf2 = mkbuf("buf2")
            buf3 = mkbuf("buf3")
            xgt = per.tile([P, GC, PADT], BF)
            biasb = per.tile([P, 3 * GC], F32)
            btb = per.tile([P, 2 * KC], F32)
            maskm = per.tile([P, PADT], BF)
            maskq = per.tile([P, PADT], BF)
            identb = per.tile([P, P], BF)
            c = per.tile([P, KC, NCB], F32)

            # initial loads
            for g in range(NG):
                nc.sync.dma_start(buf1[g][:], x0_d[:, g * S:(g + 1) * S, :])
            for l in range(3):
                nc.sync.dma_start(biasb[:, l * GC:(l + 1) * GC], bias_d[l])
            for li in range(2):
                nc.sync.dma_start(btb[:, li * KC:(li + 1) * KC], bt_d[li])
            nc.sync.dma_start(maskm[:], mm_d[:])
            nc.sync.dma_start(maskq[:], mq_d[:])
            nc.sync.dma_start(identb[:], id_d[:])
            # zero pads of the two reusable buffers (buf1 pads come from host)
            for buf in (buf2, buf3):
                for g in range(NG):
                    nc.vector.memset(buf[g][:, :, 0:PAD], 0.0)
                    nc.vector.memset(buf[g][:, :, PAD + TB:], 0.0)
            # xgt pads: warmup steps read (junk) columns there; keep finite
            nc.vector.memset(xgt[:, :, 0:PAD], 0.0)
            nc.vector.memset(xgt[:, :, PAD + TB:], 0.0)

            def bk(buf, k):
                """(part, local chunk index) for global d-chunk k."""
                return buf[k // S], k % S

            def load_w(pool, src, shape, tag):
                t_ = pool.tile(shape, BF, tag=tag)
                nc.sync.dma_start(t_[:], src)
                return t_

            def phase_xg(l, xin, wxb):
                for gp in range(GC):
                    for n in range(NCH):
                        psum = psbig.tile([P, NSZ], F32, tag="big")
                        for k in range(KC):
                            xpart, kl = bk(xin, k)
                            nc.tensor.matmul(
                                psum[:], wxb[:, k, gp, :],
                                xpart[:, kl, PAD + n * NSZ:PAD + (n + 1) * NSZ],
                                start=(k == 0), stop=(k == KC - 1))
                        nc.scalar.activation(
                            xgt[:, gp, PAD + n * NSZ:PAD + (n + 1) * NSZ],
                            psum[:],
                            AF.Identity, bias=biasb[:, l * GC + gp:l * GC + gp + 1])

            def fold_mask():
                # Fold the (t < len) mask of the reversed layer into its i/f
                # gate pre-activations: x' = x*m + (m-1)*15, i.e. unmasked
                # columns keep x, masked columns become -15 (= 0.5*(-30)),
                # which saturates sig() to exactly 0 in the tail: c and h
                # stay exactly 0 through masked steps -- replaces the
                # per-step c-mask multiply of the recurrence.
                for rows in (slice(2, 6), slice(10, 14)):
                    seg = xgt[:, rows, :]
                    mb = maskm.unsqueeze(1).to_broadcast([P, 4, PADT])
                    qb = maskq.unsqueeze(1).to_broadcast([P, 4, PADT])
                    nc.vector.tensor_tensor(seg, seg, mb, OP.mult)
                    nc.vector.tensor_tensor(seg, seg, qb, OP.add)

            MMORD = int(os.environ.get("BASSLSTM_MMORD", "0"))

            def mm_order():
                # The per-step dependency cycle is h_g(t-1) -> MMs gated by
                # it -> psum complete -> tail chain -> h_g(t).  MMORD=1:
                # group-0's full psum completes after 32 MMs (k<2 for its
                # gate chunks first, then k>=2 which need group-1's h of the
                # previous step -- ready earlier than group-0's h), so its
                # tail starts while group-1's MMs stream.  MMORD=0: all k<2
                # first (baseline order).
                if MMORD == 2:     # group-major
                    return ([(gp, k) for g in range(NG) for k in range(KC)
                             for gp in range(g * GPG, (g + 1) * GPG)],
                            (0, 1))
                if MMORD == 1:
                    order = []
                    for g in range(NG):
                        order += [(gp, k) for k in range(KC // 2)
                                  for gp in range(g * GPG, (g + 1) * GPG)]
                        order += [(gp, k) for k in range(KC // 2, KC)
                                  for gp in range(g * GPG, (g + 1) * GPG)]
                    return order, (0, 1)
                first = [(gp, k) for k in range(KC // 2) for gp in range(GC)]
                rest = []
                for g in (0, 1):
                    rest += [(gp, k)
                             for gp in range(g * GPG, (g + 1) * GPG)
                             for k in range(KC // 2, KC)]
                return first + rest, (0, 1)

            MM_ORDER, TAIL_ORDER = mm_order()

            GW = KC * BC  # mask columns per time step
            NOMM = os.environ.get("BASSLSTM_NOMM") == "1"
            NOTAIL = os.environ.get("BASSLSTM_NOTAIL") == "1"
            WARM = int(os.environ.get("BASSLSTM_WARM", "0"))
            TAILPRIO = os.environ.get("BASSLSTM_TAILPRIO", "1") == "1"

            import contextlib

            def tail_prio(l, u, gi):
                # Pseudo-time floor so the Tile scheduler lays the tail ops
                # out on the DVE/ACT FIFOs strictly in (step, group) order;
                # otherwise group 1's psum-add (ready only at stream end)
                # gets queued AHEAD of group 0's remaining chain and blocks
                # it (in-order engine queues).
                if not TAILPRIO:
                    return contextlib.nullcontext()
                return tc.tile_wait_until((l * 1000 + u * NG + gi) * 0.01)

            # h ping-pong: static-address feedback buffers so the matmul
            # stream has NO dynamic APs (kills the per-instruction pointer-
            # update sequencer work).  Split per (parity, group) for precise
            # dependency tracking.  The time-indexed output sequence is
            # archived separately on the (otherwise idle) GpSimd engine.
            hb = [[per.tile([P, S, NCB], BF, name=f"hb{p_}_{g_}")
                   for g_ in range(NG)] for p_ in range(2)]
            def rec_step(l, yout, rev, whb, W, s_expr, u, archive):
                """One recurrence step.  s_expr: affine loop expr for the
                xg/archive column base of THIS step (already in column
                units, PAD included); u: unroll parity index."""
                rd = hb[u % 2]          # h(t-1), static APs
                wr = hb[(u + 1) % 2]    # h(t)
                pss = [psb.tile([P, GPG, NCB], F32, tag="psb",
                                name=f"ps{g_}")
                       for g_ in range(NG)]
                # identity matmul seeds each group's psum with xg (+bias),
                # so the tail's tanh reads PSUM directly (no DVE add).
                # Reads xgt straight through a dynamic AP: the moving
                # operand streams by column, so the strided free dims cost
                # nothing, and xgt is only written once per layer.
                for g_ in range(NG):
                    nc.tensor.matmul(
                        pss[g_][:], identb[:],
                        xgt[:, g_ * GPG:(g_ + 1) * GPG, ds(s_expr, NCB)],
                        start=True, stop=False, skip_group_check=True)
                for pi, (gp, k) in enumerate([] if NOMM else MM_ORDER):
                    g, gl = divmod(gp, GPG)
                    rhs = rd[k // S][:, k % S, :]
                    nc.tensor.matmul(
                        pss[g][:, gl, :], whb[:, k, gp, :],
                        rhs,
                        start=False,
                        stop=(k == KC - 1),
                        skip_group_check=True)
                # Tail per group (state is tracked doubled: the hb
                # buffers hold h2 = 2h — compensated by an extra
                # 0.5 folded into Wh host-side — and c holds C' = 2c):
                #   th   = tanh(psum')          (j full, i/f/o half)
                #   t1'  = (th_i + 1) * th_j    = 2*sig_i*tanh_j
                #   u    = (th_f + 1) * C'      = 4*sig_f*c
                #   C'   = 0.5*u + t1'          = 2*c_new
                #   tct  = tanh(0.5*C')         = tanh(c_new)
                #   h2   = (th_o + 1) * tct     = 2*h
                for gi, g in enumerate([] if NOTAIL else TAIL_ORDER):
                  with tail_prio(l, u, gi):
                    th = work.tile([P, 4 * S, NCB], F32, tag="th")
                    nc.scalar.activation(th[:], pss[g][:], AF.Tanh)
                    t1 = work.tile([P, S, NCB], F32, tag="t1")
                    nc.vector.scalar_tensor_tensor(
                        t1[:], th[:, S:2 * S, :], 1.0, th[:, 0:S, :],
                        OP.add, OP.mult)
                    cg = c[:, g * S:(g + 1) * S, :]
                    uu = work.tile([P, S, NCB], F32, tag="uu")
                    nc.vector.scalar_tensor_tensor(
                        uu[:], th[:, 2 * S:3 * S, :], 1.0, cg,
                        OP.add, OP.mult)
                    nc.vector.scalar_tensor_tensor(
                        cg, uu[:], 0.5, t1[:], OP.mult, OP.add)
                    tct = work.tile([P, S, NCB], F32, tag="tct")
                    nc.scalar.activation(tct[:], cg, AF.Tanh,
                                         scale=0.5)
                    nc.vector.scalar_tensor_tensor(
                        wr[g][:], th[:, 3 * S:4 * S, :], 1.0, tct[:],
                        OP.add, OP.mult)
                    if archive:
                        # archive h2(t) = 2*h(t): contiguous NCB columns
                        # in the interleaved layout, via the (idle) DMA
                        # engines -- GpSimd dynamic-dest copies cost ~1.9us
                        # and stall concurrent DVE ops (port contention).
                        # yout holds DOUBLED values; consumers compensate
                        # (weight scales host-side + 0.5 in the highway
                        # STT).
                        nc.sync.dma_start(
                            yout[g][:, :, ds(s_expr, NCB)], wr[g][:])

            def zero_chunk(jz):
                """Exact reset of chunk jz's state (its t crossed the
                sequence boundary): zero its c and h columns."""
                sl = slice(jz * BC, (jz + 1) * BC)
                nc.vector.memset(c[:, :, sl], 0.0)
                for g_ in range(NG):
                    nc.vector.memset(hb[0][g_][:, :, sl], 0.0)

            def phase_rec(l, yout, whb, rev):
                W = WARMUP[l]
                nc.vector.memset(c[:], 0.0)
                for g_ in range(NG):
                    nc.vector.memset(hb[0][g_][:], 0.0)

                def colbase(s):
                    # column base (incl PAD) of step s's gather/archive:
                    # delta = s-W (fwd) or L-1-(s-W) (rev); the C chunks'
                    # columns are contiguous: PAD + (delta%L)*NCB +
                    # (delta//L)*BC, with delta//L constant inside a block.
                    delta = (s - W) if not rev else (L - 1 - (s - W))
                    return PAD + (delta % L) * NCB + (delta // L) * BC

                blocks = ([W % L] if W % L else []) + [L] * (W // L) + [L]
                s0 = 0
                for bi, blk in enumerate(blocks):
                    archive = (bi == len(blocks) - 1)
                    sgn = 1 if not rev else -1
                    base0 = colbase(s0)
                    with tc.For_i(0, blk // U, 1,
                                  hint_engines=(mybir.EngineType.PE,)) as i:
                        for u in range(U):
                            # affine column expr for step s = s0 + i*U + u
                            sexp = i * (sgn * U * NCB) + (
                                base0 + sgn * u * NCB)
                            rec_step(l, yout, rev, whb, W,
                                     sexp, u, archive)
                    s0 += blk
                    if s0 <= W and (W - s0) % L == 0 and (W - s0) // L < C:
                        jz = (W - s0) // L
                        zero_chunk(jz if not rev else C - 1 - jz)
                # fill yout's front pad with y(j*L - 1) for the highway
                # prev_h shift: block (L-1, j-1) -> pad cols [BC, NCB)
                # (j=0 keeps zeros = prev_h at t=0).
                for g_ in range(NG):
                    nc.gpsimd.tensor_copy(
                        yout[g_][:, :, BC:NCB],
                        yout[g_][:, :, PAD + (L - 1) * NCB:
                                 PAD + (L - 1) * NCB + (NCB - BC)])

            def phase_hw(li, y, x, out_sbuf, wtb, wcb):
                for gc_ in range(KC):
                    for n in range(NCH):
                        pt = psbig.tile([P, NSZ], F32, tag="big")
                        for k in range(KC):
                            yp, kl = bk(y, k)
                            nc.tensor.matmul(
                                pt[:], wtb[:, k, gc_, :],
                                yp[:, kl,
                                   PAD + n * NSZ - NCB:PAD + (n + 1) * NSZ - NCB],
                                start=(k == 0), stop=False)
                        for k in range(KC):
                            xp, kl = bk(x, k)
                            nc.tensor.matmul(
                                pt[:], wtb[:, KC + k, gc_, :],
                                xp[:, kl, PAD + n * NSZ:PAD + (n + 1) * NSZ],
                                start=False, stop=(k == KC - 1))
                        tg = hwork.tile([P, NSZ], BF, tag="tg")
                        nc.scalar.activation(
                            tg[:], pt[:], AF.Sigmoid,
                            bias=btb[:, li * KC + gc_:li * KC + gc_ + 1])
                        pc = psbig.tile([P, NSZ], F32, tag="big")
                        for k in range(KC):
                            xp, kl = bk(x, k)
                            nc.tensor.matmul(
                                pc[:], wcb[:, k, gc_, :],
                                xp[:, kl, PAD + n * NSZ:PAD + (n + 1) * NSZ],
                                start=(k == 0), stop=(k == KC - 1))
                        dt_ = hwork.tile([P, NSZ], F32, tag="dt")
                        ypart, ykl = bk(y, gc_)
                        # y buffers hold 2*h; fold the 0.5 into the fused op
                        nc.vector.scalar_tensor_tensor(
                            dt_[:], ypart[:, ykl, PAD + n * NSZ:PAD + (n + 1) * NSZ],
                            0.5, pc[:], OP.mult, OP.subtract)
                        nc.vector.tensor_tensor(dt_[:], dt_[:], tg[:], OP.mult)
                        if out_sbuf is not None:
                            opart, okl = bk(out_sbuf, gc_)
                            nc.vector.tensor_tensor(
                                opart[:, okl, PAD + n * NSZ:PAD + (n + 1) * NSZ],
                                dt_[:], pc[:], OP.add)
                        else:
                            st = hwork.tile([P, NSZ], F32, tag="st")
                            nc.vector.tensor_tensor(st[:], dt_[:], pc[:], OP.add)
                            nc.sync.dma_start(
                                out_d[:, gc_, n * NSZ:(n + 1) * NSZ], st[:])

            NL = int(os.environ.get("BASSLSTM_LAYERS", "3"))

            # ---- layer 1 (plain forward LSTM)
            wxb = load_w(wpool, wx_d[0], [P, KC, GC, P], "wx")
            whb = load_w(wpool, wh_d[0], [P, KC, GC, P], "wh")
            phase_xg(0, buf1, wxb)
            phase_rec(0, buf2, whb, rev=False)
            for _rep in range(int(os.environ.get("BASSLSTM_REPEAT", "1")) - 1):
                phase_rec(0, buf2, whb, rev=False)
            if NL == 1:
                for gc_ in range(KC):
                    for n in range(NCH):
                        st0 = hwork.tile([P, NSZ], F32, tag="st", name="st0")
                        bp, bkl = bk(buf2, gc_)
                        nc.vector.tensor_scalar_mul(
                            st0[:],
                            bp[:, bkl, PAD + n * NSZ:PAD + (n + 1) * NSZ],
                            0.5)
                        nc.sync.dma_start(
                            out_d[:, gc_, n * NSZ:(n + 1) * NSZ], st0[:])
            else:
                wxb2 = load_w(wpool, wx_d[1], [P, KC, GC, P], "wx")
                wtb = load_w(wpool, wt_d[0], [P, 2 * KC, KC, P], "wt")
                wcb = load_w(wpool, wc_d[0], [P, KC, KC, P], "wc")
                whb2 = load_w(wpool, wh_d[1], [P, KC, GC, P], "wh")

                # ---- layer 2 (reversed LSTM + highway)
                phase_xg(1, buf2, wxb2)
                fold_mask()
                phase_rec(1, buf3, whb2, rev=True)
                if NL == 2:
                    phase_hw(0, buf3, buf2, None, wtb, wcb)
                else:
                    wxb3 = load_w(wpool, wx_d[2], [P, KC, GC, P], "wx")
                    whb3 = load_w(wpool, wh_d[2], [P, KC, GC, P], "wh")
                    phase_hw(0, buf3, buf2, buf1, wtb, wcb)
                    wtb2 = load_w(wpool, wt_d[1], [P, 2 * KC, KC, P], "wt")
                    wcb2 = load_w(wpool, wc_d[1], [P, KC, KC, P], "wc")

                    # ---- layer 3 (forward LSTM + highway -> DRAM)
                    phase_xg(2, buf1, wxb3)
                    phase_rec(2, buf2, whb3, rev=False)
                    phase_hw(1, buf2, buf1, None, wtb2, wcb2)

    nc.compile()
    return nc


def _program():
    global _PROG
    if _PROG is None:
        _PROG = _build_program()
    return _PROG


# ---------------------------------------------------------------- host side
def _prep_weights(inp):
    """Build the shared (replicated) weight arrays in device layout."""
    gtbl = _gprime_table()
    wx = np.zeros((3, P, KC, GC, P), np.float32)
    wh = np.zeros((3, P, KC, GC, P), np.float32)
    bias = np.zeros((3, P, GC), np.float32)
    for l, (wxn, whn, bn) in enumerate(
            [("Wx1", "Wh1", "b1"), ("Wx2", "Wh2", "b2"), ("Wx3", "Wh3", "b3")]):
        Wx = np.asarray(inp[wxn], np.float32)
        Wh = np.asarray(inp[whn], np.float32)
        b = np.asarray(inp[bn], np.float32)
        for gp, (go, m) in enumerate(gtbl):
            cs = go * H + m * P
            # sigmoid gates (i,f,o) are computed as (tanh(x/2)+1)/2 on
            # device so one ACT op covers all four gates; fold the /2 into
            # the weights and bias here.  j (go==1) stays unscaled.  Wh
            # gets an extra 0.5 because the recurrent h buffer holds 2h.
            # Wx of layer 2 gets 0.5 too: its input (layer 1's archived
            # output) is doubled; layer 3's input is a highway output
            # (true scale).
            s_ = 1.0 if go == 1 else 0.5
            xs_ = 0.5 if l == 1 else 1.0
            for k in range(KC):
                wx[l, :, k, gp, :] = Wx[k * P:(k + 1) * P, cs:cs + P] * (
                    s_ * xs_)
                wh[l, :, k, gp, :] = Wh[k * P:(k + 1) * P, cs:cs + P] * (
                    s_ * 0.5)
            bias[l, :, gp] = b[cs:cs + P]
            if go == 2:  # forget gate: fold forget_bias = 1.0
                bias[l, :, gp] += 1.0
            bias[l, :, gp] *= s_
    wt = np.zeros((2, P, 2 * KC, KC, P), np.float32)
    wc = np.zeros((2, P, KC, KC, P), np.float32)
    bt = np.zeros((2, P, KC), np.float32)
    for li, (wtn, wcn, btn) in enumerate(
            [("Wt2", "Wc2", "bt2"), ("Wt3", "Wc3", "bt3")]):
        Wt = np.asarray(inp[wtn], np.float32)
        Wc = np.asarray(inp[wcn], np.float32)
        btv = np.asarray(inp[btn], np.float32)
        for gc_ in range(KC):
            cs = gc_ * P
            for k in range(2 * KC):
                # rows [0,KC): prev_h operand = this layer's archived LSTM
                # output (doubled) -> 0.5.  rows [KC,2KC): the x operand —
                # doubled for hw2 (x = layer-1 archive), true for hw3
                # (x = hw2's highway output).
                ts_ = 0.5 if (k < KC or li == 0) else 1.0
                wt[li, :, k, gc_, :] = Wt[k * P:(k + 1) * P, cs:cs + P] * ts_
            for k in range(KC):
                cs_ = 0.5 if li == 0 else 1.0
                wc[li, :, k, gc_, :] = Wc[k * P:(k + 1) * P, cs:cs + P] * cs_
            bt[li, :, gc_] = btv[cs:cs + P]
    return (wx.astype(BF16), wh.astype(BF16), wt.astype(BF16),
            wc.astype(BF16), bias, bt)


def _interleave_cols(arr_t_last):
    """[..., T, BC] -> [..., TB] in chunk-interleaved column order
    col(t=j*L+lt, b) = lt*NCB + j*BC + b."""
    shp = arr_t_last.shape[:-2]
    a = arr_t_last.reshape(*shp, C, L, BC)
    a = np.moveaxis(a, -3, -2)               # [..., L, C, BC]
    return np.ascontiguousarray(a).reshape(*shp, TB)


def _host_prep(inputs):
    x = np.asarray(inputs["inputs"], np.float32)
    length = np.asarray(inputs["length"], np.int32)
    wx, wh, wt, wc, bias, bt = _prep_weights(inputs)
    ident = np.eye(P, dtype=BF16)
    in_maps = []
    for ci in range(NCORES):
        xc = x[ci * BC:(ci + 1) * BC, :T]          # [BC, T, D]
        arr = np.ascontiguousarray(xc.transpose(2, 1, 0))  # [D, T, BC]
        arr = arr.reshape(KC, P, T, BC).transpose(1, 0, 2, 3)  # [P, KC, T, BC]
        x0t = np.zeros((P, KC, PADT), BF16)
        x0t[:, :, PAD:PAD + TB] = _interleave_cols(arr).astype(BF16)
        lc = length[ci * BC:(ci + 1) * BC]
        m = (np.arange(T)[:, None] < lc[None, :]).astype(np.float32)  # [T, BC]
        mi = _interleave_cols(m)                   # [TB]
        maskm = np.zeros((PADT,), np.float32)
        maskm[PAD:PAD + TB] = mi
        maskq = np.zeros((PADT,), np.float32)
        maskq[PAD:PAD + TB] = (mi - 1.0) * 15.0
        in_maps.append({
            "x0t": x0t,
            "wx": wx, "wh": wh, "wt": wt, "wc": wc,
            "bias": bias, "bt": bt,
            "maskm": np.broadcast_to(maskm, (P, PADT)).astype(BF16),
            "maskq": np.broadcast_to(maskq, (P, PADT)).astype(BF16),
            "ident": ident,
        })
    return in_maps


def _host_post(results, inputs):
    length = np.asarray(inputs["length"], np.int32)
    out = np.zeros((B, T, D), np.float32)
    for ci, res in enumerate(results):
        o = res["outt"]                      # [P, KC, TB] interleaved cols
        o = o.reshape(P, KC, L, C, BC)
        o = o.transpose(4, 3, 2, 1, 0)       # [BC, C, L, KC, P]
        out[ci * BC:(ci + 1) * BC] = o.reshape(BC, T, D)
    tmask = np.arange(T)[None, :] < length[:, None]
    out *= tmask[:, :, None]
    return out


_RUNNER = None


def _make_runner():
    """Build a cached shard_map-jitted executable for the 8-core program.

    Modeled on concourse.bass2jax.run_bass_via_pjrt, but reusable across
    calls and able to take pre-placed (device-resident) inputs so pure
    execution can be timed without host->device transfer.
    """
    import jax
    import numpy as jnp_np  # noqa: F401
    import concourse.mybir as mybir
    from concourse import bass2jax
    from jax.sharding import Mesh, PartitionSpec
    from jax.experimental.shard_map import shard_map

    nc = _program()
    bass2jax.install_neuronx_cc_hook()

    partition_name = (nc.partition_id_tensor.name
                      if nc.partition_id_tensor else None)
    in_names, out_names, out_avals, zero_outs = [], [], [], []
    for alloc in nc.m.functions[0].allocations:
        if not isinstance(alloc, mybir.MemoryLocationSet):
            continue
        name = alloc.memorylocations[0].name
        if alloc.kind == "ExternalInput":
            if name != partition_name:
                in_names.append(name)
        elif alloc.kind == "ExternalOutput":
            shape = tuple(alloc.tensor_shape)
            dtype = mybir.dt.np(alloc.dtype)
            out_names.append(name)
            out_avals.append(jax.core.ShapedArray(shape, dtype))
            zero_outs.append(np.zeros(shape, dtype))
    n_params = len(in_names)
    all_names = in_names + out_names
    if partition_name is not None:
        all_names.append(partition_name)

    def _body(*args):
        operands = list(args)
        if partition_name is not None:
            operands.append(bass2jax.partition_id_tensor())
        outs = bass2jax._bass_exec_p.bind(
            *operands,
            out_avals=tuple(out_avals),
            in_names=tuple(all_names),
            out_names=tuple(out_names),
            lowering_input_output_aliases=(),
            sim_require_finite=True,
            sim_require_nnan=True,
            nc=nc,
        )
        return tuple(outs)

    devices = jax.devices()[:NCORES]
    mesh = Mesh(np.asarray(devices), ("core",))
    n_out = len(out_names)
    sharded = jax.jit(
        shard_map(_body, mesh=mesh,
                  in_specs=(PartitionSpec("core"),) * (n_params + n_out),
                  out_specs=(PartitionSpec("core"),) * n_out,
                  check_rep=False),
        keep_unused=True,
    )
    return {
        "fn": sharded, "in_names": in_names, "out_names": out_names,
        "zero_outs": zero_outs, "n_params": n_params, "mesh": mesh,
    }


def _runner():
    global _RUNNER
    if _RUNNER is None:
        _RUNNER = _make_runner()
    return _RUNNER


def place_inputs(in_maps):
    """Concatenate per-core inputs on axis 0 (shard_map layout)."""
    r = _runner()
    concat = [np.concatenate([np.asarray(in_maps[c][n])
                              for c in range(NCORES)], axis=0)
              for n in r["in_names"]]
    concat += [np.zeros((NCORES * z.shape[0], *z.shape[1:]), z.dtype)
               for z in r["zero_outs"]]
    return concat


def exec_placed_nofetch(placed):
    """Run and block, but leave outputs on device (for timing)."""
    r = _runner()
    outs = r["fn"](*placed)
    for o in outs:
        o.block_until_ready()
    return outs


def exec_placed(placed):
    r = _runner()
    outs = r["fn"](*placed)
    outs = [o.block_until_ready() for o in outs]
    results = []
    for c in range(NCORES):
        m = {}
        for i, name in enumerate(r["out_names"]):
            z = r["zero_outs"][i]
            arr = np.asarray(outs[i])
            m[name] = arr[c * z.shape[0]:(c + 1) * z.shape[0]]
        results.append(m)
    return results


def run_device(in_maps):
    return exec_placed(place_inputs(in_maps))


def kernel(**inputs):
    in_maps = _host_prep(inputs)
    results = run_device(in_maps)
    return _host_post(results, inputs)



# revision 22
# speedup vs baseline: 1.9386x; 1.0296x over previous
"""Trainium2 Bass kernel for the 3-layer LSTM highway encoder.

Problem: nn_Encoding_layer (B=32, T=512, D=H=512)
  net = lstm1(x)                          # forward LSTM
  net = hw2(net)   = rev-LSTM + highway   # reversed LSTM (per-length) + highway
  net = hw3(net)   = fwd-LSTM + highway

Sharding: data-parallel, batch 32 -> 8 cores x 4 sequences. Weights replicated.

Chunked recurrence: within a core, each layer's T-step recurrence is split
into C=8 time chunks processed SIMULTANEOUSLY as extra batch columns (the
moving operand grows from BC=4 to C*BC=32 columns).  Chunk j covers
t in [j*L+W, (j+1)*L+W) (chunk 0: [0, L+W)) with L = (T-W)/C; every chunk
starts from zero state and runs L+W steps, the first W being warmup whose
outputs are later overwritten by the owning chunk (the LSTM state is
contractive, so after W steps the zero-init error is below threshold; W is
per-layer, larger for layers 2/3 whose state decays slower).  At step s
chunk j processes t = j*L + s (reverse layer: t = j*L + (L+W-1-s)), so the
per-step gather across chunks is one stepped dynamic slice of stride L.
This cuts serial steps ~3x and amortizes all per-step fixed costs 8x.

Device design (per core, everything SBUF-resident, bf16 matmuls / fp32 state):
  - Activations live transposed: [128 d-partitions, 4 d-chunks, PAD+T*4+PAD cols]
    column index = t*4 + b. Zero pads at both ends make the t=0 (forward) and
    t=T-1 (backward) steps read zero recurrent state with no special cases.
  - Phase A (per layer): xg = x @ Wx + b as 256 dense matmuls (Wx stationary
    tiles, activations moving), bias folded via ACT Identity, output bf16
    "xgt" [128, 16 gate-chunks, T*4], gate-chunk order is group/gate-major.
  - Phase B (per layer): T sequential steps in a For_i hardware loop.
    Per step: 64 matmuls (Wh [128,128] bf16 stationary tiles, moving h_{t-1}
    [128,4]) accumulating into per-group PSUM; vector tail computes
    c = sig(f)*c + sig(i)*tanh(j), h = sig(o)*tanh(c).  h lives in a static
    ping-pong buffer (hb) so every matmul AP is static (no per-instruction
    pointer-update sequencer work); GpSimd archives h into the time-indexed
    output buffer.  Matmuls are group-major so group 0's tail hides under
    group 1's matmuls and group 1's tail under the next step's group-0 ones.
    Layer 2 runs t backwards and multiplies c by a host-built (t<len) mask,
    which reproduces tf.reverse_sequence + dynamic_rnn masking exactly.
  - Phase C (layers 2,3): highway gate tg = sigmoid([prev_h, x] @ Wt + bt),
    out = y*tg + (x@Wc)*(1-tg). prev_h is just a 4-column-shifted slice.
  - Final (t >= len) output masking is done on host.
"""

import os

import ml_dtypes
import numpy as np

BF16 = ml_dtypes.bfloat16

# ---------------------------------------------------------------- constants
B, D, H = 32, 512, 512
T = int(os.environ.get("BASSLSTM_T", "512"))
NCORES = 8
BC = B // NCORES            # 4 sequences per core
P = 128
KC = D // P                 # 4 d-chunks
GC = 4 * H // P             # 16 gate chunks
NG = 2                      # hidden-chunk groups in the recurrence tail
S = KC // NG                # hidden chunks per group
GPG = GC // NG              # gate chunks per group
TB = T * BC                 # columns per d-chunk

# --- chunked recurrence ---
# Uniform chunks: C chunks of L = T/C time steps; storage column for time t
# is col(t) = (t mod L)*NCB + (t//L)*BC + b  ("chunk-interleaved" layout),
# so the per-step gather/archive across all C chunks is ONE contiguous
# NCB-column slice.  Each chunk runs W warmup steps before its L owned
# steps; chunks whose warmup window crosses t=0 (fwd; t=T-1 rev) get their
# state zeroed exactly when crossing, making them exact.
C = 8                       # time chunks per core (extra batch columns)
NCB = C * BC                # moving columns per recurrence step
L = T // C                  # owned steps per chunk
PAD = NCB                   # pad columns in activation buffers
PADT = PAD + TB + PAD
NSZ = min(512, TB)          # moving free-dim per phase-A/C matmul
NCH = TB // NSZ             # tb chunks
U = 16                      # step unroll inside For_i

_WDEF = (32, 160, 160) if T == 512 else (T // 8, T // 4, T // 4)
WARMUP = tuple(
    max(U, (int(os.environ.get(f"BASSLSTM_W{i+1}", _WDEF[i])) // U) * U)
    for i in range(3))

# g' (gate-chunk) order: groups of hidden chunks, gate-major inside a group:
# [j j .. | i i .. | f f .. | o o ..] per group.  orig TF gate order: i,j,f,o.
_GATES = (1, 0, 2, 3)       # j, i, f, o -> index into the 4H axis


def _gprime_table():
    tbl = []
    for g in range(NG):
        for go in _GATES:
            for s in range(S):
                tbl.append((go, g * S + s))
    return tbl


# ---------------------------------------------------------------- program
_PROG = None


def _build_program():
    import concourse.mybir as mybir
    import concourse.tile as tile
    from concourse import bacc
    from concourse.bass import ds

    F32 = mybir.dt.float32
    BF = mybir.dt.bfloat16
    AF = mybir.ActivationFunctionType
    OP = mybir.AluOpType

    nc = bacc.Bacc("TRN2", target_bir_lowering=False, debug=False,
                   num_devices=NCORES)

    x0_d = nc.dram_tensor("x0t", [P, KC, PADT], BF, kind="ExternalInput")
    wx_d = nc.dram_tensor("wx", [3, P, KC, GC, P], BF, kind="ExternalInput")
    wh_d = nc.dram_tensor("wh", [3, P, KC, GC, P], BF, kind="ExternalInput")
    wt_d = nc.dram_tensor("wt", [2, P, 2 * KC, KC, P], BF, kind="ExternalInput")
    wc_d = nc.dram_tensor("wc", [2, P, KC, KC, P], BF, kind="ExternalInput")
    bias_d = nc.dram_tensor("bias", [3, P, GC], F32, kind="ExternalInput")
    bt_d = nc.dram_tensor("bt", [2, P, KC], F32, kind="ExternalInput")
    # mask planes (chunk-interleaved cols): m in {0,1}; q = (m-1)*15
    mm_d = nc.dram_tensor("maskm", [P, PADT], BF, kind="ExternalInput")
    mq_d = nc.dram_tensor("maskq", [P, PADT], BF, kind="ExternalInput")
    id_d = nc.dram_tensor("ident", [P, P], BF, kind="ExternalInput")
    out_d = nc.dram_tensor("outt", [P, KC, TB], F32, kind="ExternalOutput")

    with tile.TileContext(nc) as tc:
        with (
            tc.tile_pool(name="per", bufs=1) as per,
            tc.tile_pool(name="wpool", bufs=1) as wpool,
            tc.tile_pool(name="work", bufs=6) as work,
            tc.tile_pool(name="hwork", bufs=3) as hwork,
            tc.tile_pool(name="psb", bufs=5, space="PSUM") as psb,
            tc.tile_pool(name="psbig", bufs=3, space="PSUM") as psbig,
        ):
            # Activation buffers are split into NG parts (hidden-chunk
            # groups).  Cross-step deps through these tensors use dynamic
            # APs, which Tile tracks at whole-tensor granularity — the split
            # makes next step's k<S matmuls depend only on group 0's h-write,
            # hiding group 1's vector tail under the next step's matmuls.
            def mkbuf(name):
                return tuple(
                    per.tile([P, S, PADT], BF, name=f"{name}_{g}")
                    for g in range(NG))

            buf1 = mkbuf("buf1")
            buf2 = mkbuf("buf2")
            buf3 = mkbuf("buf3")
            xgt = per.tile([P, GC, PADT], BF)
            biasb = per.tile([P, 3 * GC], F32)
            btb = per.tile([P, 2 * KC], F32)
            maskm = per.tile([P, PADT], BF)
            maskq = per.tile([P, PADT], BF)
            identb = per.tile([P, P], BF)
            c = per.tile([P, KC, NCB], F32)

            # initial loads
            for g in range(NG):
                nc.sync.dma_start(buf1[g][:], x0_d[:, g * S:(g + 1) * S, :])
            for l in range(3):
                nc.sync.dma_start(biasb[:, l * GC:(l + 1) * GC], bias_d[l])
            for li in range(2):
                nc.sync.dma_start(btb[:, li * KC:(li + 1) * KC], bt_d[li])
            nc.sync.dma_start(maskm[:], mm_d[:])
            nc.sync.dma_start(maskq[:], mq_d[:])
            nc.sync.dma_start(identb[:], id_d[:])
            # zero pads of the two reusable buffers (buf1 pads come from host)
            for buf in (buf2, buf3):
                for g in range(NG):
                    nc.vector.memset(buf[g][:, :, 0:PAD], 0.0)
                    nc.vector.memset(buf[g][:, :, PAD + TB:], 0.0)
            # xgt pads: warmup steps read (junk) columns there; keep finite
            nc.vector.memset(xgt[:, :, 0:PAD], 0.0)
            nc.vector.memset(xgt[:, :, PAD + TB:], 0.0)

            def bk(buf, k):
                """(part, local chunk index) for global d-chunk k."""
                return buf[k // S], k % S

            def load_w(pool, src, shape, tag):
                t_ = pool.tile(shape, BF, tag=tag)
                nc.sync.dma_start(t_[:], src)
                return t_

            def phase_xg(l, xin, wxb):
                for gp in range(GC):
                    for n in range(NCH):
                        psum = psbig.tile([P, NSZ], F32, tag="big")
                        for k in range(KC):
                            xpart, kl = bk(xin, k)
                            nc.tensor.matmul(
                                psum[:], wxb[:, k, gp, :],
                                xpart[:, kl, PAD + n * NSZ:PAD + (n + 1) * NSZ],
                                start=(k == 0), stop=(k == KC - 1))
                        nc.scalar.activation(
                            xgt[:, gp, PAD + n * NSZ:PAD + (n + 1) * NSZ],
                            psum[:],
                            AF.Identity, bias=biasb[:, l * GC + gp:l * GC + gp + 1])

            def fold_mask():
                # Fold the (t < len) mask of the reversed layer into its i/f
                # gate pre-activations: x' = x*m + (m-1)*15, i.e. unmasked
                # columns keep x, masked columns become -15 (= 0.5*(-30)),
                # which saturates sig() to exactly 0 in the tail: c and h
                # stay exactly 0 through masked steps -- replaces the
                # per-step c-mask multiply of the recurrence.
                for rows in (slice(2, 6), slice(10, 14)):
                    seg = xgt[:, rows, :]
                    mb = maskm.unsqueeze(1).to_broadcast([P, 4, PADT])
                    qb = maskq.unsqueeze(1).to_broadcast([P, 4, PADT])
                    nc.vector.tensor_tensor(seg, seg, mb, OP.mult)
                    nc.vector.tensor_tensor(seg, seg, qb, OP.add)

            MMORD = int(os.environ.get("BASSLSTM_MMORD", "0"))

            def mm_order():
                # The per-step dependency cycle is h_g(t-1) -> MMs gated by
                # it -> psum complete -> tail chain -> h_g(t).  MMORD=1:
                # group-0's full psum completes after 32 MMs (k<2 for its
                # gate chunks first, then k>=2 which need group-1's h of the
                # previous step -- ready earlier than group-0's h), so its
                # tail starts while group-1's MMs stream.  MMORD=0: all k<2
                # first (baseline order).
                if MMORD == 2:     # group-major
                    return ([(gp, k) for g in range(NG) for k in range(KC)
                             for gp in range(g * GPG, (g + 1) * GPG)],
                            (0, 1))
                if MMORD == 1:
                    order = []
                    for g in range(NG):
                        order += [(gp, k) for k in range(KC // 2)
                                  for gp in range(g * GPG, (g + 1) * GPG)]
                        order += [(gp, k) for k in range(KC // 2, KC)
                                  for gp in range(g * GPG, (g + 1) * GPG)]
                    return order, (0, 1)
                first = [(gp, k) for k in range(KC // 2) for gp in range(GC)]
                rest = []
                for g in (0, 1):
                    rest += [(gp, k)
                             for gp in range(g * GPG, (g + 1) * GPG)
                             for k in range(KC // 2, KC)]
                return first + rest, (0, 1)

            MM_ORDER, TAIL_ORDER = mm_order()

            GW = KC * BC  # mask columns per time step
            NOMM = os.environ.get("BASSLSTM_NOMM") == "1"
            NOTAIL = os.environ.get("BASSLSTM_NOTAIL") == "1"
            WARM = int(os.environ.get("BASSLSTM_WARM", "0"))
            TAILPRIO = os.environ.get("BASSLSTM_TAILPRIO", "1") == "1"

            import contextlib

            def tail_prio(l, u, gi):
                # Pseudo-time floor so the Tile scheduler lays the tail ops
                # out on the DVE/ACT FIFOs strictly in (step, group) order;
                # otherwise group 1's psum-add (ready only at stream end)
                # gets queued AHEAD of group 0's remaining chain and blocks
                # it (in-order engine queues).
                if not TAILPRIO:
                    return contextlib.nullcontext()
                return tc.tile_wait_until((l * 1000 + u * NG + gi) * 0.01)

            # h ping-pong: static-address feedback buffers so the matmul
            # stream has NO dynamic APs (kills the per-instruction pointer-
            # update sequencer work).  Split per (parity, group) for precise
            # dependency tracking.  The time-indexed output sequence is
            # archived separately on the (otherwise idle) GpSimd engine.
            hb = [[per.tile([P, S, NCB], BF, name=f"hb{p_}_{g_}")
                   for g_ in range(NG)] for p_ in range(2)]
            def rec_step(l, yout, rev, whb, W, s_expr, u, archive):
                """One recurrence step.  s_expr: affine loop expr for the
                xg/archive column base of THIS step (already in column
                units, PAD included); u: unroll parity index."""
                rd = hb[u % 2]          # h(t-1), static APs
                wr = hb[(u + 1) % 2]    # h(t)
                pss = [psb.tile([P, GPG, NCB], F32, tag="psb",
                                name=f"ps{g_}")
                       for g_ in range(NG)]
                # identity matmul seeds each group's psum with xg (+bias),
                # so the tail's tanh reads PSUM directly (no DVE add).
                # Reads xgt straight through a dynamic AP: the moving
                # operand streams by column, so the strided free dims cost
                # nothing, and xgt is only written once per layer.
                for g_ in range(NG):
                    nc.tensor.matmul(
                        pss[g_][:], identb[:],
                        xgt[:, g_ * GPG:(g_ + 1) * GPG, ds(s_expr, NCB)],
                        start=True, stop=False, skip_group_check=True)
                for pi, (gp, k) in enumerate([] if NOMM else MM_ORDER):
                    g, gl = divmod(gp, GPG)
                    rhs = rd[k // S][:, k % S, :]
                    nc.tensor.matmul(
                        pss[g][:, gl, :], whb[:, k, gp, :],
                        rhs,
                        start=False,
                        stop=(k == KC - 1),
                        skip_group_check=True)
                # Tail per group (state is tracked doubled: the hb
                # buffers hold h2 = 2h — compensated by an extra
                # 0.5 folded into Wh host-side — and c holds C' = 2c):
                #   th   = tanh(psum')          (j full, i/f/o half)
                #   t1'  = (th_i + 1) * th_j    = 2*sig_i*tanh_j
                #   u    = (th_f + 1) * C'      = 4*sig_f*c
                #   C'   = 0.5*u + t1'          = 2*c_new
                #   tct  = tanh(0.5*C')         = tanh(c_new)
                #   h2   = (th_o + 1) * tct     = 2*h
                for gi, g in enumerate([] if NOTAIL else TAIL_ORDER):
                  with tail_prio(l, u, gi):
                    th = work.tile([P, 4 * S, NCB], F32, tag="th")
                    nc.scalar.activation(th[:], pss[g][:], AF.Tanh)
                    t1 = work.tile([P, S, NCB], F32, tag="t1")
                    nc.vector.scalar_tensor_tensor(
                        t1[:], th[:, S:2 * S, :], 1.0, th[:, 0:S, :],
                        OP.add, OP.mult)
                    cg = c[:, g * S:(g + 1) * S, :]
                    uu = work.tile([P, S, NCB], F32, tag="uu")
                    nc.vector.scalar_tensor_tensor(
                        uu[:], th[:, 2 * S:3 * S, :], 1.0, cg,
                        OP.add, OP.mult)
                    nc.vector.scalar_tensor_tensor(
                        cg, uu[:], 0.5, t1[:], OP.mult, OP.add)
                    tct = work.tile([P, S, NCB], F32, tag="tct")
                    nc.scalar.activation(tct[:], cg, AF.Tanh,
                                         scale=0.5)
                    nc.vector.scalar_tensor_tensor(
                        wr[g][:], th[:, 3 * S:4 * S, :], 1.0, tct[:],
                        OP.add, OP.mult)
                    if archive:
                        # archive h2(t) = 2*h(t): contiguous NCB columns
                        # in the interleaved layout, via the (idle) DMA
                        # engines -- GpSimd dynamic-dest copies cost ~1.9us
                        # and stall concurrent DVE ops (port contention).
                        # yout holds DOUBLED values; consumers compensate
                        # (weight scales host-side + 0.5 in the highway
                        # STT).
                        nc.sync.dma_start(
                            yout[g][:, :, ds(s_expr, NCB)], wr[g][:])

            def zero_chunk(jz):
                """Exact reset of chunk jz's state (its t crossed the
                sequence boundary): zero its c and h columns."""
                sl = slice(jz * BC, (jz + 1) * BC)
                nc.vector.memset(c[:, :, sl], 0.0)
                for g_ in range(NG):
                    nc.vector.memset(hb[0][g_][:, :, sl], 0.0)

            def phase_rec(l, yout, whb, rev):
                W = WARMUP[l]
                nc.vector.memset(c[:], 0.0)
                for g_ in range(NG):
                    nc.vector.memset(hb[0][g_][:], 0.0)

                def colbase(s):
                    # column base (incl PAD) of step s's gather/archive:
                    # delta = s-W (fwd) or L-1-(s-W) (rev); the C chunks'
                    # columns are contiguous: PAD + (delta%L)*NCB +
                    # (delta//L)*BC, with delta//L constant inside a block.
                    delta = (s - W) if not rev else (L - 1 - (s - W))
                    return PAD + (delta % L) * NCB + (delta // L) * BC

                blocks = ([W % L] if W % L else []) + [L] * (W // L) + [L]
                s0 = 0
                for bi, blk in enumerate(blocks):
                    archive = (bi == len(blocks) - 1)
                    sgn = 1 if not rev else -1
                    base0 = colbase(s0)
                    with tc.For_i(0, blk // U, 1,
                                  hint_engines=(mybir.EngineType.PE,)) as i:
                        for u in range(U):
                            # affine column expr for step s = s0 + i*U + u
                            sexp = i * (sgn * U * NCB) + (
                                base0 + sgn * u * NCB)
                            rec_step(l, yout, rev, whb, W,
                                     sexp, u, archive)
                    s0 += blk
                    if s0 <= W and (W - s0) % L == 0 and (W - s0) // L < C:
                        jz = (W - s0) // L
                        zero_chunk(jz if not rev else C - 1 - jz)
                # fill yout's front pad with y(j*L - 1) for the highway
                # prev_h shift: block (L-1, j-1) -> pad cols [BC, NCB)
                # (j=0 keeps zeros = prev_h at t=0).
                for g_ in range(NG):
                    nc.gpsimd.tensor_copy(
                        yout[g_][:, :, BC:NCB],
                        yout[g_][:, :, PAD + (L - 1) * NCB:
                                 PAD + (L - 1) * NCB + (NCB - BC)])

            def phase_hw(li, y, x, out_sbuf, wtb, wcb):
                for gc_ in range(KC):
                    for n in range(NCH):
                        pt = psbig.tile([P, NSZ], F32, tag="big")
                        for k in range(KC):
                            yp, kl = bk(y, k)
                            nc.tensor.matmul(
                                pt[:], wtb[:, k, gc_, :],
                                yp[:, kl,
                                   PAD + n * NSZ - NCB:PAD + (n + 1) * NSZ - NCB],
                                start=(k == 0), stop=False)
                        for k in range(KC):
                            xp, kl = bk(x, k)
                            nc.tensor.matmul(
                                pt[:], wtb[:, KC + k, gc_, :],
                                xp[:, kl, PAD + n * NSZ:PAD + (n + 1) * NSZ],
                                start=False, stop=(k == KC - 1))
                        tg = hwork.tile([P, NSZ], BF, tag="tg")
                        nc.scalar.activation(
                            tg[:], pt[:], AF.Sigmoid,
                            bias=btb[:, li * KC + gc_:li * KC + gc_ + 1])
                        pc = psbig.tile([P, NSZ], F32, tag="big")
                        for k in range(KC):
                            xp, kl = bk(x, k)
                            nc.tensor.matmul(
                                pc[:], wcb[:, k, gc_, :],
                                xp[:, kl, PAD + n * NSZ:PAD + (n + 1) * NSZ],
                                start=(k == 0), stop=(k == KC - 1))
                        dt_ = hwork.tile([P, NSZ], F32, tag="dt")
                        ypart, ykl = bk(y, gc_)
                        # y buffers hold 2*h; fold the 0.5 into the fused op
                        nc.vector.scalar_tensor_tensor(
                            dt_[:], ypart[:, ykl, PAD + n * NSZ:PAD + (n + 1) * NSZ],
                            0.5, pc[:], OP.mult, OP.subtract)
                        nc.vector.tensor_tensor(dt_[:], dt_[:], tg[:], OP.mult)
                        if out_sbuf is not None:
                            opart, okl = bk(out_sbuf, gc_)
                            nc.vector.tensor_tensor(
                                opart[:, okl, PAD + n * NSZ:PAD + (n + 1) * NSZ],
                                dt_[:], pc[:], OP.add)
                        else:
                            st = hwork.tile([P, NSZ], F32, tag="st")
                            nc.vector.tensor_tensor(st[:], dt_[:], pc[:], OP.add)
                            nc.sync.dma_start(
                                out_d[:, gc_, n * NSZ:(n + 1) * NSZ], st[:])

            NL = int(os.environ.get("BASSLSTM_LAYERS", "3"))

            # ---- layer 1 (plain forward LSTM)
            wxb = load_w(wpool, wx_d[0], [P, KC, GC, P], "wx")
            whb = load_w(wpool, wh_d[0], [P, KC, GC, P], "wh")
            phase_xg(0, buf1, wxb)
            phase_rec(0, buf2, whb, rev=False)
            for _rep in range(int(os.environ.get("BASSLSTM_REPEAT", "1")) - 1):
                phase_rec(0, buf2, whb, rev=False)
            if NL == 1:
                for gc_ in range(KC):
                    for n in range(NCH):
                        st0 = hwork.tile([P, NSZ], F32, tag="st", name="st0")
                        bp, bkl = bk(buf2, gc_)
                        nc.vector.tensor_scalar_mul(
                            st0[:],
                            bp[:, bkl, PAD + n * NSZ:PAD + (n + 1) * NSZ],
                            0.5)
                        nc.sync.dma_start(
                            out_d[:, gc_, n * NSZ:(n + 1) * NSZ], st0[:])
            else:
                wxb2 = load_w(wpool, wx_d[1], [P, KC, GC, P], "wx")
                wtb = load_w(wpool, wt_d[0], [P, 2 * KC, KC, P], "wt")
                wcb = load_w(wpool, wc_d[0], [P, KC, KC, P], "wc")
                whb2 = load_w(wpool, wh_d[1], [P, KC, GC, P], "wh")

                # ---- layer 2 (reversed LSTM + highway)
                phase_xg(1, buf2, wxb2)
                fold_mask()
                phase_rec(1, buf3, whb2, rev=True)
                if NL == 2:
                    phase_hw(0, buf3, buf2, None, wtb, wcb)
                else:
                    wxb3 = load_w(wpool, wx_d[2], [P, KC, GC, P], "wx")
                    whb3 = load_w(wpool, wh_d[2], [P, KC, GC, P], "wh")
                    phase_hw(0, buf3, buf2, buf1, wtb, wcb)
                    wtb2 = load_w(wpool, wt_d[1], [P, 2 * KC, KC, P], "wt")
                    wcb2 = load_w(wpool, wc_d[1], [P, KC, KC, P], "wc")

                    # ---- layer 3 (forward LSTM + highway -> DRAM)
                    phase_xg(2, buf1, wxb3)
                    phase_rec(2, buf2, whb3, rev=False)
                    phase_hw(1, buf2, buf1, None, wtb2, wcb2)

    nc.compile()
    return nc


def _program():
    global _PROG
    if _PROG is None:
        _PROG = _build_program()
    return _PROG


# ---------------------------------------------------------------- host side
def _prep_weights(inp):
    """Build the shared (replicated) weight arrays in device layout."""
    gtbl = _gprime_table()
    wx = np.zeros((3, P, KC, GC, P), np.float32)
    wh = np.zeros((3, P, KC, GC, P), np.float32)
    bias = np.zeros((3, P, GC), np.float32)
    for l, (wxn, whn, bn) in enumerate(
            [("Wx1", "Wh1", "b1"), ("Wx2", "Wh2", "b2"), ("Wx3", "Wh3", "b3")]):
        Wx = np.asarray(inp[wxn], np.float32)
        Wh = np.asarray(inp[whn], np.float32)
        b = np.asarray(inp[bn], np.float32)
        for gp, (go, m) in enumerate(gtbl):
            cs = go * H + m * P
            # sigmoid gates (i,f,o) are computed as (tanh(x/2)+1)/2 on
            # device so one ACT op covers all four gates; fold the /2 into
            # the weights and bias here.  j (go==1) stays unscaled.  Wh
            # gets an extra 0.5 because the recurrent h buffer holds 2h.
            # Wx of layer 2 gets 0.5 too: its input (layer 1's archived
            # output) is doubled; layer 3's input is a highway output
            # (true scale).
            s_ = 1.0 if go == 1 else 0.5
            xs_ = 0.5 if l == 1 else 1.0
            for k in range(KC):
                wx[l, :, k, gp, :] = Wx[k * P:(k + 1) * P, cs:cs + P] * (
                    s_ * xs_)
                wh[l, :, k, gp, :] = Wh[k * P:(k + 1) * P, cs:cs + P] * (
                    s_ * 0.5)
            bias[l, :, gp] = b[cs:cs + P]
            if go == 2:  # forget gate: fold forget_bias = 1.0
                bias[l, :, gp] += 1.0
            bias[l, :, gp] *= s_
    wt = np.zeros((2, P, 2 * KC, KC, P), np.float32)
    wc = np.zeros((2, P, KC, KC, P), np.float32)
    bt = np.zeros((2, P, KC), np.float32)
    for li, (wtn, wcn, btn) in enumerate(
            [("Wt2", "Wc2", "bt2"), ("Wt3", "Wc3", "bt3")]):
        Wt = np.asarray(inp[wtn], np.float32)
        Wc = np.asarray(inp[wcn], np.float32)
        btv = np.asarray(inp[btn], np.float32)
        for gc_ in range(KC):
            cs = gc_ * P
            for k in range(2 * KC):
                # rows [0,KC): prev_h operand = this layer's archived LSTM
                # output (doubled) -> 0.5.  rows [KC,2KC): the x operand —
                # doubled for hw2 (x = layer-1 archive), true for hw3
                # (x = hw2's highway output).
                ts_ = 0.5 if (k < KC or li == 0) else 1.0
                wt[li, :, k, gc_, :] = Wt[k * P:(k + 1) * P, cs:cs + P] * ts_
            for k in range(KC):
                cs_ = 0.5 if li == 0 else 1.0
                wc[li, :, k, gc_, :] = Wc[k * P:(k + 1) * P, cs:cs + P] * cs_
            bt[li, :, gc_] = btv[cs:cs + P]
    return (wx.astype(BF16), wh.astype(BF16), wt.astype(BF16),
            wc.astype(BF16), bias, bt)


def _interleave_cols(arr_t_last):
    """[..., T, BC] -> [..., TB] in chunk-interleaved column order
    col(t=j*L+lt, b) = lt*NCB + j*BC + b."""
    shp = arr_t_last.shape[:-2]
    a = arr_t_last.reshape(*shp, C, L, BC)
    a = np.moveaxis(a, -3, -2)               # [..., L, C, BC]
    return np.ascontiguousarray(a).reshape(*shp, TB)


def _host_prep(inputs):
    x = np.asarray(inputs["inputs"], np.float32)
    length = np.asarray(inputs["length"], np.int32)
    wx, wh, wt, wc, bias, bt = _prep_weights(inputs)
    ident = np.eye(P, dtype=BF16)
    in_maps = []
    for ci in range(NCORES):
        xc = x[ci * BC:(ci + 1) * BC, :T]          # [BC, T, D]
        arr = np.ascontiguousarray(xc.transpose(2, 1, 0))  # [D, T, BC]
        arr = arr.reshape(KC, P, T, BC).transpose(1, 0, 2, 3)  # [P, KC, T, BC]
        x0t = np.zeros((P, KC, PADT), BF16)
        x0t[:, :, PAD:PAD + TB] = _interleave_cols(arr).astype(BF16)
        lc = length[ci * BC:(ci + 1) * BC]
        m = (np.arange(T)[:, None] < lc[None, :]).astype(np.float32)  # [T, BC]
        mi = _interleave_cols(m)                   # [TB]
        maskm = np.zeros((PADT,), np.float32)
        maskm[PAD:PAD + TB] = mi
        maskq = np.zeros((PADT,), np.float32)
        maskq[PAD:PAD + TB] = (mi - 1.0) * 15.0
        in_maps.append({
            "x0t": x0t,
            "wx": wx, "wh": wh, "wt": wt, "wc": wc,
            "bias": bias, "bt": bt,
            "maskm": np.broadcast_to(maskm, (P, PADT)).astype(BF16),
            "maskq": np.broadcast_to(maskq, (P, PADT)).astype(BF16),
            "ident": ident,
        })
    return in_maps


def _host_post(results, inputs):
    length = np.asarray(inputs["length"], np.int32)
    out = np.zeros((B, T, D), np.float32)
    for ci, res in enumerate(results):
        o = res["outt"]                      # [P, KC, TB] interleaved cols
        o = o.reshape(P, KC, L, C, BC)
        o = o.transpose(4, 3, 2, 1, 0)       # [BC, C, L, KC, P]
        out[ci * BC:(ci + 1) * BC] = o.reshape(BC, T, D)
    tmask = np.arange(T)[None, :] < length[:, None]
    out *= tmask[:, :, None]
    return out


_RUNNER = None


def _make_runner():
    """Build a cached shard_map-jitted executable for the 8-core program.

    Modeled on concourse.bass2jax.run_bass_via_pjrt, but reusable across
    calls and able to take pre-placed (device-resident) inputs so pure
    execution can be timed without host->device transfer.
    """
    import jax
    import numpy as jnp_np  # noqa: F401
    import concourse.mybir as mybir
    from concourse import bass2jax
    from jax.sharding import Mesh, PartitionSpec
    from jax.experimental.shard_map import shard_map

    nc = _program()
    bass2jax.install_neuronx_cc_hook()

    partition_name = (nc.partition_id_tensor.name
                      if nc.partition_id_tensor else None)
    in_names, out_names, out_avals, zero_outs = [], [], [], []
    for alloc in nc.m.functions[0].allocations:
        if not isinstance(alloc, mybir.MemoryLocationSet):
            continue
        name = alloc.memorylocations[0].name
        if alloc.kind == "ExternalInput":
            if name != partition_name:
                in_names.append(name)
        elif alloc.kind == "ExternalOutput":
            shape = tuple(alloc.tensor_shape)
            dtype = mybir.dt.np(alloc.dtype)
            out_names.append(name)
            out_avals.append(jax.core.ShapedArray(shape, dtype))
            zero_outs.append(np.zeros(shape, dtype))
    n_params = len(in_names)
    all_names = in_names + out_names
    if partition_name is not None:
        all_names.append(partition_name)

    def _body(*args):
        operands = list(args)
        if partition_name is not None:
            operands.append(bass2jax.partition_id_tensor())
        outs = bass2jax._bass_exec_p.bind(
            *operands,
            out_avals=tuple(out_avals),
            in_names=tuple(all_names),
            out_names=tuple(out_names),
            lowering_input_output_aliases=(),
            sim_require_finite=True,
            sim_require_nnan=True,
            nc=nc,
        )
        return tuple(outs)

    devices = jax.devices()[:NCORES]
    mesh = Mesh(np.asarray(devices), ("core",))
    n_out = len(out_names)
    sharded = jax.jit(
        shard_map(_body, mesh=mesh,
                  in_specs=(PartitionSpec("core"),) * (n_params + n_out),
                  out_specs=(PartitionSpec("core"),) * n_out,
                  check_rep=False),
        keep_unused=True,
    )
    return {
        "fn": sharded, "in_names": in_names, "out_names": out_names,
        "zero_outs": zero_outs, "n_params": n_params, "mesh": mesh,
    }


def _runner():
    global _RUNNER
    if _RUNNER is None:
        _RUNNER = _make_runner()
    return _RUNNER


def place_inputs(in_maps):
    """Concatenate per-core inputs on axis 0 (shard_map layout)."""
    r = _runner()
    concat = [np.concatenate([np.asarray(in_maps[c][n])
                              for c in range(NCORES)], axis=0)
              for n in r["in_names"]]
    concat += [np.zeros((NCORES * z.shape[0], *z.shape[1:]), z.dtype)
               for z in r["zero_outs"]]
    return concat


def exec_placed_nofetch(placed):
    """Run and block, but leave outputs on device (for timing)."""
    r = _runner()
    outs = r["fn"](*placed)
    for o in outs:
        o.block_until_ready()
    return outs


def exec_placed(placed):
    r = _runner()
    outs = r["fn"](*placed)
    outs = [o.block_until_ready() for o in outs]
    results = []
    for c in range(NCORES):
        m = {}
        for i, name in enumerate(r["out_names"]):
            z = r["zero_outs"][i]
            arr = np.asarray(outs[i])
            m[name] = arr[c * z.shape[0]:(c + 1) * z.shape[0]]
        results.append(m)
    return results


def run_device(in_maps):
    return exec_placed(place_inputs(in_maps))


def kernel(**inputs):
    in_maps = _host_prep(inputs)
    results = run_device(in_maps)
    return _host_post(results, inputs)

